# revision 7
# baseline (speedup 1.0000x reference)
"""Trainium2 Bass kernel for an AxialAttentionLayer-style module.

Math: for each batch b,
    scores = q @ k'          where k'[e,j] = keys[e,j] + sum_d keys[j,d]
    A      = softmax(scores, axis=-1)
    out    = A @ values
(the reference's rank-1 additive score s1 folds into the matmul because
 s1[l,j] = (sum_e q[l,e]) * ksum[j] = sum_e q[l,e]*ksum[j]).

Sharding: data-parallel over batch B=32 across 8 cores (4 batches/core).
Device layout per core, per 512-row block of L:
    MM1   (PE, fp32):   scores(l,s) = qT_chunk.T @ k'   (4x 128-tiles)
    max   (DVE):        fused 3D reduce_max(negate) -> -rowmax (128,4)
    exp   (ACT):        P = exp(scores - rowmax), per-tile bias
    rowsum(DVE):        fused 3D reduce_sum over P -> staged per batch
    T     (PE):         P chunks transposed via PE transpose -> PSUM
    copy  (ACT):        PT PSUM -> SBUF (rounded to fp32r)
    MM2   (PE, fp32r):  outT(d,l) = v.T-free matmul with v stationary
    copy  (DVE):        outT PSUM -> SBUF
Host: pre-transposes q -> qT, builds k', divides by rowsum, transposes back.
"""

import numpy as np

B, L, S = 32, 8192, 128
N_CORES = 8
B_LOC = B // N_CORES  # 4
LBLK = 512            # l-rows per block
NT = LBLK // 128      # 128-tiles per block
NBLK = L // LBLK      # blocks per batch

_RUNNER_CACHE = {}

# tunables (overridable before building)
CFG = dict(
    qt_bufs=8, p_bufs=4, pt_bufs=4, nm_bufs=4, oc_bufs=6, rs_bufs=2,
    sc_bufs=4, ptps_bufs=2, o_bufs=2,
    out_copy_engine="dve",   # "act" | "dve" | "alt"
    pt_copy_engine="act",    # "act" | "dve" | "alt"
    store_engine="sp",       # "pool" | "act" | "dve" | "sp"
    rowsum_mode="dve",       # "dve" | "pool" | "pe"
    mm1_dtype="f32",         # "f32" | "f32r" | "bf16x2"
    out_dtype="f32",         # "f32" | "bf16"
    bias_mode="act",         # "act" (per-tile exp bias) | "pe" (K=4 accum matmul)
    nm_copy_engine="act",    # "dve" | "act"
    scs_copy_engine="dve",   # "dve" | "act"
    dataflow="t2",           # "lsoft" | "t" | "t2" (fp16 MM1 + host shift)
    rsps_bufs=1,
    max_out_dtype="f32r",    # partition_all_reduce out dtype in "t" flow
    # --- t2 knobs ---
    t2_rowsum="pe",          # "pe" (sel16 matmul) | "pool" (gpsimd all-reduce)
    t2_oc_engine="dve",      # "dve" | "act"
    t2_rs_copy="dve",        # "dve" | "act" | "pool"
    t2_sc_bufs=3, t2_o_bufs=2, t2_rsps_bufs=2,
)


def _build_nc_t2(repeat=1, cfg=None):
    """fp16 MM1 + K=4 bf16 additive (rank-1 score + host softmax shift),
    no device max, bf16 P/V/out. Layout: scores in (j part, l free)."""
    cfg = {**CFG, **(cfg or {})}
    import concourse.bacc as bacc
    import concourse.mybir as mybir
    import concourse.tile as tile
    from concourse.bass import ts
    from concourse import bass_isa

    f32 = mybir.dt.float32
    f16 = mybir.dt.float16
    bf16 = mybir.dt.bfloat16
    Exp = mybir.ActivationFunctionType.Exp

    nc = bacc.Bacc("TRN2", target_bir_lowering=False, debug=False)
    qT_d = nc.dram_tensor("qT16", (B_LOC, S, L), f16, kind="ExternalInput")
    k_d = nc.dram_tensor("k16", (B_LOC, S, S), f16, kind="ExternalInput")
    v_d = nc.dram_tensor("v16", (B_LOC, S, S), bf16, kind="ExternalInput")
    alhs_d = nc.dram_tensor("alhs", (B_LOC, 4, S), bf16, kind="ExternalInput")
    arhs_d = nc.dram_tensor("arhs", (B_LOC, 4, L), bf16, kind="ExternalInput")
    sel_d = nc.dram_tensor("selb", (S, NBLK * NBLK), bf16, kind="ExternalInput")
    outT_d = nc.dram_tensor("outT", (B_LOC, S, L), bf16, kind="ExternalOutput")
    rs_d = nc.dram_tensor("rs", (B_LOC, NBLK, LBLK), f32, kind="ExternalOutput")

    with tile.TileContext(nc) as tc:
        with (
            tc.tile_pool(name="const", bufs=1) as constp,
            tc.tile_pool(name="qt", bufs=cfg["qt_bufs"]) as qtp,
            tc.tile_pool(name="ra", bufs=4) as rap,
            tc.tile_pool(name="pt", bufs=cfg["pt_bufs"]) as ptp,
            tc.tile_pool(name="oc", bufs=cfg["oc_bufs"]) as ocp,
            tc.tile_pool(name="rss", bufs=cfg["rs_bufs"]) as rsp,
            tc.tile_pool(name="scps", bufs=cfg["t2_sc_bufs"], space="PSUM") as scps,
            tc.tile_pool(name="ops", bufs=cfg["t2_o_bufs"], space="PSUM") as ops,
            tc.tile_pool(name="rsps", bufs=cfg["t2_rsps_bufs"], space="PSUM") as rspsp,
        ):
            k_sb = constp.tile([128, B_LOC * 128], f16, tag="k16")
            v_sb = constp.tile([128, B_LOC * 128], bf16, tag="v16")
            alhs_sb = constp.tile([4, B_LOC * 128], bf16, tag="alhs")
            sel_sb = None
            if cfg["t2_rowsum"] == "pe":
                sel_sb = constp.tile([128, NBLK * NBLK], bf16, tag="selb")
                nc.sync.dma_start(sel_sb[:], sel_d[:])
            for b in range(B_LOC):
                nc.sync.dma_start(k_sb[:, ts(b, 128)], k_d[b])
                nc.sync.dma_start(v_sb[:, ts(b, 128)], v_d[b])
                nc.sync.dma_start(alhs_sb[:, ts(b, 128)], alhs_d[b])

            def t2_block(b, blk, holder):
                l0 = blk * LBLK
                qt = qtp.tile([128, LBLK], f16, tag="qt")
                nc.sync.dma_start(qt[:], qT_d[b, :, l0:l0 + LBLK])
                ra = rap.tile([4, LBLK], bf16, tag="ra")
                nc.sync.dma_start(ra[:], arhs_d[b, :, l0:l0 + LBLK])
                sc = scps.tile([128, LBLK], f32, tag="sc")
                nc.tensor.matmul(sc[:], k_sb[:, ts(b, 128)], qt[:],
                                 start=True, stop=False)
                nc.tensor.matmul(sc[:], alhs_sb[:, ts(b, 128)], ra[:],
                                 start=False, stop=True,
                                 skip_group_check=True)
                pt = ptp.tile([128, LBLK], bf16, tag="pt")
                nc.scalar.activation(pt[:], sc[:], Exp, bias=0.0, scale=1.0)
                if cfg["t2_rowsum"] == "pe":
                    if blk == 0:
                        rs16_new = rspsp.tile([NBLK, LBLK], f32, tag="rsps")
                        holder[0] = rs16_new
                    rs16 = holder[0]
                    nc.tensor.matmul(
                        rs16[:], sel_sb[:, blk * NBLK:(blk + 1) * NBLK],
                        pt[:], start=(blk == 0), stop=(blk == NBLK - 1))
                    if blk == NBLK - 1:
                        rsx = rsp.tile([NBLK, LBLK], f32, tag="rsx")
                        if cfg["t2_rs_copy"] == "act":
                            nc.scalar.copy(rsx[:], rs16[:])
                        else:
                            nc.vector.tensor_copy(rsx[:], rs16[:])
                        nc.sync.dma_start(rs_d[b], rsx[:])
                else:  # pool: gpsimd all-reduce per block
                    prt = rsp.tile([128, LBLK], f32, tag="prt")
                    nc.gpsimd.partition_all_reduce(
                        prt[:], pt[:], 128, bass_isa.ReduceOp.add)
                    nc.sync.dma_start(
                        rs_d[b, blk], prt[0:1, :].rearrange("a b -> (a b)"))
                op_t = ops.tile([128, LBLK], f32, tag="op")
                nc.tensor.matmul(op_t[:], v_sb[:, ts(b, 128)], pt[:],
                                 start=True, stop=True)
                oc = ocp.tile([128, LBLK], bf16, tag="oc")
                if cfg["t2_oc_engine"] == "act":
                    nc.scalar.copy(oc[:], op_t[:])
                else:
                    nc.vector.tensor_copy(oc[:], op_t[:])
                nc.sync.dma_start(outT_d[b, :, l0:l0 + LBLK], oc[:])

            def body(_iv=None):
                for b in range(B_LOC):
                    holder = [None]
                    for blk in range(NBLK):
                        t2_block(b, blk, holder)

            if repeat == 1:
                body()
            else:
                with tc.For_i(0, repeat, 1) as _i:
                    body(_i)

    nc.compile()
    return nc


def _build_nc(repeat=1, cfg=None):
    cfg = {**CFG, **(cfg or {})}
    if cfg["dataflow"] == "t2":
        return _build_nc_t2(repeat, cfg)
    import concourse.bacc as bacc
    import concourse.mybir as mybir
    import concourse.tile as tile
    from concourse.bass import ts
    from concourse.masks import make_identity

    f32 = mybir.dt.float32
    f32r = mybir.dt.float32r

    nc = bacc.Bacc("TRN2", target_bir_lowering=False, debug=False)
    bf16 = mybir.dt.bfloat16
    if cfg["mm1_dtype"] == "bf16x2":
        qT_d = nc.dram_tensor("qT2", (B_LOC, S, 2, L), bf16, kind="ExternalInput")
        kph_d = nc.dram_tensor("kph", (B_LOC, S, S), bf16, kind="ExternalInput")
        kpl_d = nc.dram_tensor("kpl", (B_LOC, S, S), bf16, kind="ExternalInput")
    else:
        mm1_dt_glob = f32 if cfg["mm1_dtype"] == "f32" else f32r
        qT_d = nc.dram_tensor("qT", (B_LOC, S, L), mm1_dt_glob, kind="ExternalInput")
    kp_d = nc.dram_tensor("kp", (B_LOC, S, S), f32, kind="ExternalInput")
    v_d = nc.dram_tensor("v", (B_LOC, S, S), f32, kind="ExternalInput")
    ind_d = None
    if cfg["bias_mode"] == "pe":
        ind_d = nc.dram_tensor("ind", (NT, LBLK), f32r, kind="ExternalInput")
    sel16_d = None
    if cfg["dataflow"] == "t" or cfg["rowsum_mode"] == "pe_pt":
        sel16_d = nc.dram_tensor("sel16", (S, NBLK * NBLK), f32r,
                                 kind="ExternalInput")
    out_dt = f32 if cfg["out_dtype"] == "f32" else mybir.dt.bfloat16
    outT_d = nc.dram_tensor("outT", (B_LOC, S, L), out_dt, kind="ExternalOutput")
    if cfg["rowsum_mode"] == "dve" and cfg["dataflow"] == "lsoft":
        rs_d = nc.dram_tensor("rs", (B_LOC, S, L // S), f32, kind="ExternalOutput")
    elif cfg["dataflow"] == "t" or cfg["rowsum_mode"] == "pe_pt":
        rs_d = nc.dram_tensor("rs", (B_LOC, NBLK, LBLK), f32, kind="ExternalOutput")
    else:
        rs_d = nc.dram_tensor("rs", (B_LOC, L), f32, kind="ExternalOutput")

    from concourse import bass_isa
    Exp = mybir.ActivationFunctionType.Exp
    AX = mybir.AxisListType.X
    MAX = mybir.AluOpType.max
    ADD = mybir.AluOpType.add

    with tile.TileContext(nc) as tc:
        with (
            tc.tile_pool(name="const", bufs=1) as constp,
            tc.tile_pool(name="qt", bufs=cfg["qt_bufs"]) as qtp,
            tc.tile_pool(name="p", bufs=cfg["p_bufs"]) as pp,
            tc.tile_pool(name="pt", bufs=cfg["pt_bufs"]) as ptp,
            tc.tile_pool(name="nm", bufs=cfg["nm_bufs"]) as nmp,
            tc.tile_pool(name="rss", bufs=cfg["rs_bufs"]) as rsp,
            tc.tile_pool(name="oc", bufs=cfg["oc_bufs"]) as ocp,
            tc.tile_pool(name="scps", bufs=cfg["sc_bufs"], space="PSUM") as scps,
            tc.tile_pool(name="ptps", bufs=cfg["ptps_bufs"], space="PSUM") as ptps,
            tc.tile_pool(name="ops", bufs=cfg["o_bufs"], space="PSUM") as ops,
            tc.tile_pool(name="rsps", bufs=cfg["rsps_bufs"], space="PSUM") as rspsp,
            tc.tile_pool(name="auxps", bufs=1, space="PSUM") as auxps,
        ):
            ident = constp.tile([128, 128], f32, tag="ident")
            make_identity(nc, ident[:])
            kp_sb = constp.tile([128, B_LOC * 128], f32, tag="kp")
            v_sb = constp.tile([128, B_LOC * 128], f32, tag="v")
            v_r = constp.tile([128, B_LOC * 128], f32r, tag="vr")
            ind_r = None
            if cfg["bias_mode"] == "pe":
                ind_r = constp.tile([NT, LBLK], f32r, tag="ind")
                nc.sync.dma_start(ind_r[:], ind_d[:])
            ones_r = None
            if cfg["rowsum_mode"] == "pe" or cfg["dataflow"] == "t":
                ones_f = constp.tile([128, 1], f32, tag="ones_f")
                ones_r = constp.tile([128, 1], f32r, tag="ones")
                nc.gpsimd.memset(ones_f[:], 1.0)
                nc.vector.tensor_copy(ones_r[:], ones_f[:])
            neg_inv_r = None
            sel16_r = None
            if cfg["rowsum_mode"] == "pe_pt" and cfg["dataflow"] != "t":
                sel16_r = constp.tile([128, NBLK * NBLK], f32r, tag="sel16")
                nc.sync.dma_start(sel16_r[:], sel16_d[:])
            if cfg["dataflow"] == "t":
                neg_inv_f = constp.tile([128, 128], f32, tag="ninv_f")
                neg_inv_r = constp.tile([128, 128], f32r, tag="ninv")
                nc.gpsimd.memset(neg_inv_f[:], -1.0 / 128.0)
                nc.vector.tensor_copy(neg_inv_r[:], neg_inv_f[:])
                sel16_r = constp.tile([128, NBLK * NBLK], f32r, tag="sel16")
                nc.sync.dma_start(sel16_r[:], sel16_d[:])
            for b in range(B_LOC):
                nc.sync.dma_start(kp_sb[:, ts(b, 128)], kp_d[b])
                nc.sync.dma_start(v_sb[:, ts(b, 128)], v_d[b])
            nc.vector.tensor_copy(v_r[:], v_sb[:])
            kp_r = None
            if cfg["mm1_dtype"] == "f32r":
                kp_r = constp.tile([128, B_LOC * 128], f32r, tag="kpr")
                nc.vector.tensor_copy(kp_r[:], kp_sb[:])
            kph_sb = kpl_sb = None
            if cfg["mm1_dtype"] == "bf16x2":
                bf16_ = mybir.dt.bfloat16
                kph_sb = constp.tile([128, B_LOC * 128], bf16_, tag="kph")
                kpl_sb = constp.tile([128, B_LOC * 128], bf16_, tag="kpl")
                for b in range(B_LOC):
                    nc.sync.dma_start(kph_sb[:, ts(b, 128)], kph_d[b])
                    nc.sync.dma_start(kpl_sb[:, ts(b, 128)], kpl_d[b])

            def t_block(b, blk, rs_stage, rs_ps_holder):
                l0 = blk * LBLK
                sc = scps.tile([128, LBLK], f32, tag="sc")
                if cfg["mm1_dtype"] == "bf16x2":
                    bf16_ = mybir.dt.bfloat16
                    qt2 = qtp.tile([128, 2 * LBLK], bf16_, tag="qt")
                    nc.sync.dma_start(
                        qt2[:].rearrange("p (h l) -> p h l", h=2),
                        qT_d[b, :, :, l0:l0 + LBLK])
                    qh = qt2[:, 0:LBLK]
                    ql = qt2[:, LBLK:2 * LBLK]
                    nc.tensor.matmul(sc[:], kph_sb[:, ts(b, 128)], qh,
                                     start=True, stop=False)
                    nc.tensor.matmul(sc[:], kpl_sb[:, ts(b, 128)], qh,
                                     start=False, stop=False)
                    nc.tensor.matmul(sc[:], kph_sb[:, ts(b, 128)], ql,
                                     start=False, stop=False)
                else:
                    mm1_dt = f32 if cfg["mm1_dtype"] == "f32" else f32r
                    kp_use = kp_sb if cfg["mm1_dtype"] == "f32" else kp_r
                    qt = qtp.tile([128, LBLK], mm1_dt, tag="qt")
                    nc.sync.dma_start(qt[:], qT_d[b, :, l0:l0 + LBLK])
                    nc.tensor.matmul(sc[:], kp_use[:, ts(b, 128)], qt[:],
                                     start=True, stop=False)
                scs = pp.tile([128, LBLK], f32, tag="scs")
                if cfg["scs_copy_engine"] == "dve":
                    nc.vector.tensor_copy(scs[:], sc[:])
                else:
                    nc.scalar.copy(scs[:], sc[:])
                mx_dt = f32r if cfg["max_out_dtype"] == "f32r" else f32
                mxr = ptp.tile([128, LBLK], mx_dt, tag="mxr")
                nc.gpsimd.partition_all_reduce(
                    mxr[:], scs[:], 128, bass_isa.ReduceOp.max)
                nc.tensor.matmul(sc[:], neg_inv_r[:], mxr[:],
                                 start=False, stop=True)
                pt = ptp.tile([128, LBLK], f32r, tag="pt")
                nc.scalar.activation(pt[:], sc[:], Exp, bias=0.0, scale=1.0)
                if blk == 0:
                    rs_ps_new = rspsp.tile([NBLK, LBLK], f32, tag="rsps")
                    rs_ps_holder[0] = rs_ps_new
                rs_ps = rs_ps_holder[0]
                nc.tensor.matmul(rs_ps[:], sel16_r[:, blk * NBLK:(blk + 1) * NBLK],
                                 pt[:], start=(blk == 0), stop=(blk == NBLK - 1))
                if blk == NBLK - 1:
                    rsx = rsp.tile([NBLK, LBLK], f32, tag="rsx")
                    nc.vector.tensor_copy(rsx[:], rs_ps[:])
                    nc.sync.dma_start(rs_d[b], rsx[:])
                op_t = ops.tile([128, LBLK], f32, tag="op")
                nc.tensor.matmul(op_t[:], v_r[:, ts(b, 128)], pt[:],
                                 start=True, stop=True)
                oc = ocp.tile([128, LBLK], out_dt, tag="oc")
                oce = cfg["out_copy_engine"]
                if oce in ("alt", "act") or oce.startswith("mix"):
                    nc.scalar.copy(oc[:], op_t[:])
                else:
                    nc.vector.tensor_copy(oc[:], op_t[:])
                st = {"pool": nc.gpsimd, "act": nc.scalar,
                      "dve": nc.vector, "sp": nc.sync}[cfg["store_engine"]]
                st.dma_start(outT_d[b, :, l0:l0 + LBLK], oc[:])

            def t_body(_iv=None):
                for b in range(B_LOC):
                    holder = [None]
                    for blk in range(NBLK):
                        t_block(b, blk, None, holder)

            def body(_iv=None):
                if cfg["dataflow"] == "t":
                    return t_body(_iv)
                for b in range(B_LOC):
                    mode = cfg["rowsum_mode"]
                    rs16_holder = [None]
                    rs_stage = None
                    if mode == "dve":
                        rs_stage = rsp.tile([128, L // S], f32, tag="rss")
                    elif mode == "pool":
                        rs_stage = rsp.tile([128, L], f32, tag="rss")
                    for blk in range(NBLK):
                        l0 = blk * LBLK
                        mm1_dt = f32 if cfg["mm1_dtype"] == "f32" else f32r
                        qt = qtp.tile([128, LBLK], mm1_dt, tag="qt")
                        nc.sync.dma_start(qt[:], qT_d[b, :, l0:l0 + LBLK])
                        sc = scps.tile([128, LBLK], f32, tag="sc")
                        for ti in range(NT):
                            nc.tensor.matmul(
                                sc[:, ts(ti, 128)], qt[:, ts(ti, 128)],
                                (kp_sb if cfg["mm1_dtype"] == "f32" else kp_r)[:, ts(b, 128)],
                                start=True,
                                stop=(cfg["bias_mode"] == "act"),
                                skip_group_check=(cfg["bias_mode"] == "pe"))
                        nm = nmp.tile([128, NT], f32, tag="nm")
                        nc.vector.tensor_reduce(
                            nm[:], sc[:].rearrange("p (t s) -> p t s", t=NT),
                            axis=AX, op=MAX, negate=True)
                        p = pp.tile([128, LBLK], f32, tag="p")
                        if cfg["bias_mode"] == "act":
                            for ti in range(NT):
                                nc.scalar.activation(
                                    p[:, ts(ti, 128)], sc[:, ts(ti, 128)], Exp,
                                    bias=nm[:, ti:ti + 1], scale=1.0)
                        else:
                            nmt_ps = auxps.tile([NT, 128], f32, tag="nmt")
                            nc.tensor.transpose(nmt_ps[:], nm[:], ident[:])
                            nmt = nmp.tile([NT, 128], f32r, tag="nmtr")
                            if cfg["nm_copy_engine"] == "dve":
                                nc.vector.tensor_copy(nmt[:], nmt_ps[:])
                            else:
                                nc.scalar.copy(nmt[:], nmt_ps[:])
                            nc.tensor.matmul(sc[:], nmt[:], ind_r[:],
                                             start=False, stop=True,
                                             skip_group_check=True)
                            nc.scalar.activation(p[:], sc[:], Exp,
                                                 bias=0.0, scale=1.0)
                        if cfg["rowsum_mode"] == "dve":
                            nc.vector.tensor_reduce(
                                rs_stage[:, blk * NT:(blk + 1) * NT],
                                p[:].rearrange("p (t s) -> p t s", t=NT),
                                axis=AX, op=ADD)
                        ptps_t = ptps.tile([128, LBLK], f32, tag="ptps")
                        for ti in range(NT):
                            nc.tensor.transpose(
                                ptps_t[:, ts(ti, 128)], p[:, ts(ti, 128)],
                                ident[:])
                        pt = ptp.tile([128, LBLK], f32r, tag="pt")
                        pce = cfg["pt_copy_engine"]
                        if pce == "alt":
                            pce = "dve" if blk % 2 == 0 else "act"
                        elif pce.startswith("mix"):
                            n, m = pce[3:].split("of")
                            pce = "dve" if blk % int(m) < int(n) else "act"
                        if pce == "dve":
                            nc.vector.tensor_copy(pt[:], ptps_t[:])
                        else:
                            nc.scalar.copy(pt[:], ptps_t[:])
                        if cfg["rowsum_mode"] == "pool":
                            nc.gpsimd.partition_all_reduce(
                                rs_stage[:, blk * LBLK:(blk + 1) * LBLK],
                                pt[:], 128, bass_isa.ReduceOp.add)
                        elif cfg["rowsum_mode"] == "pe":
                            if blk % 4 == 0:
                                rs_ps = rspsp.tile([128, LBLK], f32, tag="rsps")
                            j = blk % 4
                            nc.tensor.matmul(
                                rs_ps[32 * j:32 * j + 1, :], ones_r[:], pt[:],
                                start=True, stop=True,
                                tile_position=(0, 32 * j))
                            if j == 3:
                                nc.vector.tensor_copy(
                                    rs_stage[(blk - 3) // 4 * 4:(blk - 3) // 4 * 4 + 4, :].rearrange("a b -> a b"),
                                    rs_ps[:].rearrange("(a c) b -> a c b", c=32)[:, 0:1, :].rearrange("a c b -> (a c) b"))
                        if cfg["rowsum_mode"] == "pe_pt":
                            if blk == 0:
                                rs16_new = rspsp.tile([NBLK, LBLK], f32,
                                                      tag="rsps")
                                rs16_holder[0] = rs16_new
                            rs16 = rs16_holder[0]
                            nc.tensor.matmul(
                                rs16[:],
                                sel16_r[:, blk * NBLK:(blk + 1) * NBLK],
                                pt[:], start=(blk == 0),
                                stop=(blk == NBLK - 1))
                            if blk == NBLK - 1:
                                rsx = rsp.tile([NBLK, LBLK], f32, tag="rsx")
                                nc.vector.tensor_copy(rsx[:], rs16[:])
                                nc.sync.dma_start(rs_d[b], rsx[:])
                        op_t = ops.tile([128, LBLK], f32, tag="op")
                        nc.tensor.matmul(
                            op_t[:], v_r[:, ts(b, 128)], pt[:],
                            start=True, stop=True)
                        oc = ocp.tile([128, LBLK], out_dt, tag="oc")
                        oce = cfg["out_copy_engine"]
                        if oce == "alt":
                            oce = "act" if blk % 2 == 0 else "dve"
                        elif oce.startswith("mix"):
                            n, m = oce[3:].split("of")
                            oce = "dve" if blk % int(m) < int(n) else "act"
                        if oce == "act":
                            nc.scalar.copy(oc[:], op_t[:])
                        else:
                            nc.vector.tensor_copy(oc[:], op_t[:])
                        st = {"pool": nc.gpsimd, "act": nc.scalar,
                              "dve": nc.vector, "sp": nc.sync}[cfg["store_engine"]]
                        st.dma_start(outT_d[b, :, l0:l0 + LBLK], oc[:])
                    if cfg["rowsum_mode"] == "dve":
                        nc.gpsimd.dma_start(rs_d[b], rs_stage[:])
                    elif cfg["rowsum_mode"] == "pool":
                        nc.sync.dma_start(rs_d[b], rs_stage[0:1, :].rearrange("a b -> (a b)"))

            if repeat == 1:
                body()
            else:
                with tc.For_i(0, repeat, 1) as _i:
                    body(_i)

    nc.compile()
    return nc


def _make_runner(repeat=1, cfg=None):
    """Compile (once) and return fn(in_maps) -> list[dict] per core."""
    key = (repeat, tuple(sorted((cfg or {}).items())))
    if key in _RUNNER_CACHE:
        return _RUNNER_CACHE[key]

    import jax
    import concourse.mybir as mybir
    from concourse import bass2jax
    from concourse.bass2jax import _bass_exec_p, partition_id_tensor
    from jax.sharding import Mesh, NamedSharding, PartitionSpec
    from jax.experimental.shard_map import shard_map

    nc = _build_nc(repeat, cfg)
    bass2jax.install_neuronx_cc_hook()

    in_names, out_names, out_avals, zero_shapes = [], [], [], []
    for alloc in nc.m.functions[0].allocations:
        if not isinstance(alloc, mybir.MemoryLocationSet):
            continue
        name = alloc.memorylocations[0].name
        if alloc.kind == "ExternalInput":
            if nc.partition_id_tensor is None or name != nc.partition_id_tensor.name:
                in_names.append(name)
        elif alloc.kind == "ExternalOutput":
            out_names.append(name)
            shape = tuple(alloc.tensor_shape)
            dtype = mybir.dt.np(alloc.dtype)
            out_avals.append(jax.core.ShapedArray(shape, dtype))
            zero_shapes.append((shape, dtype))
    n_params = len(in_names)
    pid_name = nc.partition_id_tensor.name if nc.partition_id_tensor else None
    names_for_bind = in_names + out_names + ([pid_name] if pid_name else [])

    def _body(*args):
        operands = list(args)
        if pid_name:
            operands.append(partition_id_tensor())
        outs = _bass_exec_p.bind(
            *operands,
            out_avals=tuple(out_avals),
            in_names=tuple(names_for_bind),
            out_names=tuple(out_names),
            lowering_input_output_aliases=(),
            sim_require_finite=True,
            sim_require_nnan=True,
            nc=nc,
        )
        return tuple(outs)

    devices = jax.devices()[:N_CORES]
    mesh = Mesh(np.asarray(devices), ("core",))
    nspec = n_params + len(out_names)
    fn = jax.jit(
        shard_map(_body, mesh=mesh,
                  in_specs=(PartitionSpec("core"),) * nspec,
                  out_specs=(PartitionSpec("core"),) * len(out_names),
                  check_rep=False),
        keep_unused=True)
    sharding = NamedSharding(mesh, PartitionSpec("core"))

    def run(in_maps):
        import jax as _jax
        concat_in = [
            np.concatenate([np.asarray(m[name]) for m in in_maps], axis=0)
            for name in in_names
        ]
        zeros = [np.zeros((N_CORES * s[0],) + tuple(s[1:]), d)
                 for (s, d) in zero_shapes]
        dev_in = [_jax.device_put(a, sharding) for a in concat_in + zeros]
        out_arrs = fn(*dev_in)
        _jax.block_until_ready(out_arrs)
        return [
            {name: np.asarray(out_arrs[i]).reshape(
                (N_CORES,) + tuple(out_avals[i].shape))[c]
             for i, name in enumerate(out_names)}
            for c in range(N_CORES)
        ], (fn, dev_in)

    _RUNNER_CACHE[key] = run
    return run


def _prep_inputs_t2(queries, keys, values):
    import ml_dtypes
    bf = ml_dtypes.bfloat16
    q = np.asarray(queries, np.float32)
    k = np.asarray(keys, np.float32)
    v = np.asarray(values, np.float32)
    qT16 = np.ascontiguousarray(q.transpose(0, 2, 1).astype(np.float16))
    k16 = np.ascontiguousarray(k.astype(np.float16))           # (B, e, j)
    v16 = np.ascontiguousarray(v.astype(bf))                   # (B, j, d)
    qsum = q.sum(axis=2)                                       # (B, L) f32
    krow = k.sum(axis=2)                                       # (B, S) f32
    qh = qsum.astype(bf)
    ql = (qsum - qh.astype(np.float32)).astype(bf)
    kh = krow.astype(bf)
    kl = (krow - kh.astype(np.float32)).astype(bf)
    c = np.maximum(qsum * krow.max(axis=1)[:, None],
                   qsum * krow.min(axis=1)[:, None])           # (B, L)
    cb = c.astype(bf)
    ones = np.ones((B, S), np.float32).astype(bf)
    alhs = np.ascontiguousarray(
        np.stack([kh, kl, kh, -ones], axis=1))                 # (B, 4, S)
    arhs = np.ascontiguousarray(
        np.stack([qh, qh, ql, cb], axis=1))                    # (B, 4, L)
    sel = np.zeros((S, NBLK * NBLK), np.float32)
    for j in range(NBLK):
        sel[:, j * NBLK + j] = 1.0
    selb = sel.astype(bf)
    in_maps = []
    for ci in range(N_CORES):
        sl = slice(ci * B_LOC, (ci + 1) * B_LOC)
        in_maps.append({"qT16": qT16[sl], "k16": k16[sl], "v16": v16[sl],
                        "alhs": alhs[sl], "arhs": arhs[sl], "selb": selb})
    return in_maps


def _assemble_t2(results):
    out = np.empty((B, L, S), dtype=np.float32)
    for ci in range(N_CORES):
        outT = results[ci]["outT"]         # (B_LOC, S, L) bf16, = out^T
        rs = results[ci]["rs"]             # (B_LOC, NBLK, LBLK) f32
        for b in range(B_LOC):
            rsum = rs[b].reshape(L)
            out[ci * B_LOC + b] = outT[b].astype(np.float32).T / rsum[:, None]
    return out.reshape(B, 1, L, S)


def _prep_inputs(queries, keys, values, cfg=None):
    cfg = {**CFG, **(cfg or {})}
    if cfg["dataflow"] == "t2":
        return _prep_inputs_t2(queries, keys, values)
    qT = np.ascontiguousarray(queries.transpose(0, 2, 1))      # (B, E, L)
    kp = keys + keys.sum(axis=2)[:, None, :]                   # k' = k + 1*ksum
    kp = np.ascontiguousarray(kp.astype(np.float32))
    v = np.ascontiguousarray(values.astype(np.float32))
    qT2 = kph = kpl = None
    if cfg["mm1_dtype"] == "bf16x2":
        import ml_dtypes
        bf = ml_dtypes.bfloat16
        qTh = qT.astype(bf)
        qTl = (qT - qTh.astype(np.float32)).astype(bf)
        qT2 = np.ascontiguousarray(np.stack([qTh, qTl], axis=2))
        kph = kp.astype(bf)
        kpl = np.ascontiguousarray((kp - kph.astype(np.float32)).astype(bf))
        kph = np.ascontiguousarray(kph)
    ind = np.zeros((NT, LBLK), np.float32)
    for ti in range(NT):
        ind[ti, ti * 128:(ti + 1) * 128] = 1.0
    sel16 = np.zeros((S, NBLK * NBLK), np.float32)
    for j in range(NBLK):
        sel16[:, j * NBLK + j] = 1.0
    in_maps = []
    for c in range(N_CORES):
        sl = slice(c * B_LOC, (c + 1) * B_LOC)
        m = {"qT": qT[sl], "kp": kp[sl], "v": v[sl], "ind": ind,
             "sel16": sel16}
        if qT2 is not None:
            m.update({"qT2": qT2[sl], "kph": kph[sl], "kpl": kpl[sl]})
        in_maps.append(m)
    return in_maps


def _assemble(results, cfg=None):
    cfg = {**CFG, **(cfg or {})}
    if cfg["dataflow"] == "t2":
        return _assemble_t2(results)
    out = np.empty((B, L, S), dtype=np.float32)
    for c in range(N_CORES):
        outT = results[c]["outT"]          # (B_LOC, S, L)  = out^T per batch
        rs = results[c]["rs"]              # (B_LOC, 128, L//128) rowsums
        for b in range(B_LOC):
            if cfg["rowsum_mode"] == "dve" and cfg["dataflow"] == "lsoft":
                rsum = rs[b].T.reshape(L)  # rowsum[l]
            else:
                rsum = rs[b].reshape(L)
            out[c * B_LOC + b] = outT[b].T / rsum[:, None]
    return out.reshape(B, 1, L, S)


def kernel(queries, keys, values):
    run = _make_runner(repeat=1)
    in_maps = _prep_inputs(queries, keys, values)
    results, _ = run(in_maps)
    return _assemble(results)



# revision 11
# speedup vs baseline: 1.1900x; 1.1900x over previous
"""Trainium2 Bass kernel for an AxialAttentionLayer-style module.

Math: for each batch b,
    scores = q @ k'          where k'[e,j] = keys[e,j] + sum_d keys[j,d]
    A      = softmax(scores, axis=-1)
    out    = A @ values
(the reference's rank-1 additive score s1 folds into the matmul because
 s1[l,j] = (sum_e q[l,e]) * ksum[j] = sum_e q[l,e]*ksum[j]).

Sharding: data-parallel over batch B=32 across 8 cores (4 batches/core).
Device layout per core, per 512-row block of L:
    MM1   (PE, fp32):   scores(l,s) = qT_chunk.T @ k'   (4x 128-tiles)
    max   (DVE):        fused 3D reduce_max(negate) -> -rowmax (128,4)
    exp   (ACT):        P = exp(scores - rowmax), per-tile bias
    rowsum(DVE):        fused 3D reduce_sum over P -> staged per batch
    T     (PE):         P chunks transposed via PE transpose -> PSUM
    copy  (ACT):        PT PSUM -> SBUF (rounded to fp32r)
    MM2   (PE, fp32r):  outT(d,l) = v.T-free matmul with v stationary
    copy  (DVE):        outT PSUM -> SBUF
Host: pre-transposes q -> qT, builds k', divides by rowsum, transposes back.
"""

import numpy as np

B, L, S = 32, 8192, 128
N_CORES = 8
B_LOC = B // N_CORES  # 4
LBLK = 512            # l-rows per block
NT = LBLK // 128      # 128-tiles per block
NBLK = L // LBLK      # blocks per batch

_RUNNER_CACHE = {}

# tunables (overridable before building)
CFG = dict(
    qt_bufs=8, p_bufs=4, pt_bufs=4, nm_bufs=4, oc_bufs=6, rs_bufs=2,
    sc_bufs=4, ptps_bufs=2, o_bufs=2,
    out_copy_engine="dve",   # "act" | "dve" | "alt"
    pt_copy_engine="act",    # "act" | "dve" | "alt"
    store_engine="sp",       # "pool" | "act" | "dve" | "sp"
    rowsum_mode="dve",       # "dve" | "pool" | "pe"
    mm1_dtype="f32",         # "f32" | "f32r" | "bf16x2"
    out_dtype="f32",         # "f32" | "bf16"
    bias_mode="act",         # "act" (per-tile exp bias) | "pe" (K=4 accum matmul)
    nm_copy_engine="act",    # "dve" | "act"
    scs_copy_engine="dve",   # "dve" | "act"
    dataflow="t2",           # "lsoft" | "t" | "t2" (fp16 MM1 + host shift)
    rsps_bufs=1,
    max_out_dtype="f32r",    # partition_all_reduce out dtype in "t" flow
    # --- t2 knobs ---
    t2_rowsum="pe",          # "pe" (sel16 matmul) | "pool" (gpsimd all-reduce)
    t2_oc_engine="dve",      # "dve" | "act"
    t2_rs_copy="dve",        # "dve" | "act" | "pool"
    t2_sc_bufs=3, t2_o_bufs=2, t2_rsps_bufs=2,
    t2_qs_bufs=2, t2_ot_bufs=2, t2_q_dmas=2, t2_o_dmas=1,
)


def _build_nc_t2(repeat=1, cfg=None):
    """fp16 MM1 + K=4 bf16 additive (rank-1 score + host softmax shift),
    no device max, bf16 P/V/out. Layout: scores in (j part, l free)."""
    cfg = {**CFG, **(cfg or {})}
    import concourse.bacc as bacc
    import concourse.mybir as mybir
    import concourse.tile as tile
    from concourse.bass import ts
    from concourse import bass_isa

    f32 = mybir.dt.float32
    f16 = mybir.dt.float16
    bf16 = mybir.dt.bfloat16
    Exp = mybir.ActivationFunctionType.Exp

    nc = bacc.Bacc("TRN2", target_bir_lowering=False, debug=False)
    qT_d = nc.dram_tensor("qT16", (B_LOC, S, L), f16, kind="ExternalInput")
    k_d = nc.dram_tensor("k16", (B_LOC, S, S), f16, kind="ExternalInput")
    v_d = nc.dram_tensor("v16", (B_LOC, S, S), bf16, kind="ExternalInput")
    alhs_d = nc.dram_tensor("alhs", (B_LOC, 4, S), f16, kind="ExternalInput")
    arhs_d = nc.dram_tensor("arhs", (B_LOC, 4, L), f16, kind="ExternalInput")
    sel_d = nc.dram_tensor("selb", (S, NBLK * NBLK), bf16, kind="ExternalInput")
    outT_d = nc.dram_tensor("outT", (B_LOC, S, L), bf16, kind="ExternalOutput")
    rs_d = nc.dram_tensor("rs", (B_LOC, NBLK, LBLK), f32, kind="ExternalOutput")

    NQD = cfg["t2_q_dmas"]     # DMAs per batch for q load
    NOD = cfg["t2_o_dmas"]     # DMAs per batch for out store
    with tile.TileContext(nc) as tc:
        with (
            tc.tile_pool(name="const", bufs=1) as constp,
            tc.tile_pool(name="qs", bufs=cfg["t2_qs_bufs"]) as qsp,
            tc.tile_pool(name="ot", bufs=cfg["t2_ot_bufs"]) as otp,
            tc.tile_pool(name="pt", bufs=cfg["pt_bufs"]) as ptp,
            tc.tile_pool(name="rss", bufs=cfg["rs_bufs"]) as rsp,
            tc.tile_pool(name="scps", bufs=cfg["t2_sc_bufs"], space="PSUM") as scps,
            tc.tile_pool(name="ops", bufs=cfg["t2_o_bufs"], space="PSUM") as ops,
            tc.tile_pool(name="rsps", bufs=cfg["t2_rsps_bufs"], space="PSUM") as rspsp,
        ):
            k_sb = constp.tile([128, B_LOC * 128], f16, tag="k16")
            v_sb = constp.tile([128, B_LOC * 128], bf16, tag="v16")
            alhs_sb = constp.tile([4, B_LOC * 128], f16, tag="alhs")
            arc_sb = constp.tile([4, B_LOC * L], f16, tag="arc")
            sel_sb = constp.tile([128, NBLK * NBLK], bf16, tag="selb")
            nc.sync.dma_start(sel_sb[:], sel_d[:])
            for b in range(B_LOC):
                nc.sync.dma_start(k_sb[:, ts(b, 128)], k_d[b])
                nc.sync.dma_start(v_sb[:, ts(b, 128)], v_d[b])
                nc.sync.dma_start(alhs_sb[:, ts(b, 128)], alhs_d[b])
                nc.sync.dma_start(arc_sb[:, b * L:(b + 1) * L], arhs_d[b])

            def t2_batch(b):
                qs = qsp.tile([128, L], f16, tag="qs")
                H = L // NQD
                for h in range(NQD):
                    nc.sync.dma_start(qs[:, h * H:(h + 1) * H],
                                      qT_d[b, :, h * H:(h + 1) * H])
                ot = otp.tile([128, L], bf16, tag="ot")
                rs16 = rspsp.tile([NBLK, LBLK], f32, tag="rsps")
                for blk in range(NBLK):
                    l0 = blk * LBLK
                    sc = scps.tile([128, LBLK], f32, tag="sc")
                    nc.tensor.matmul(sc[:], k_sb[:, ts(b, 128)],
                                     qs[:, l0:l0 + LBLK],
                                     start=True, stop=False)
                    nc.tensor.matmul(sc[:], alhs_sb[:, ts(b, 128)],
                                     arc_sb[:, b * L + l0:b * L + l0 + LBLK],
                                     start=False, stop=True)
                    pt = ptp.tile([128, LBLK], bf16, tag="pt")
                    nc.scalar.activation(pt[:], sc[:], Exp, bias=0.0, scale=1.0)
                    nc.tensor.matmul(
                        rs16[:], sel_sb[:, blk * NBLK:(blk + 1) * NBLK],
                        pt[:], start=(blk == 0), stop=(blk == NBLK - 1))
                    op_t = ops.tile([128, LBLK], f32, tag="op")
                    nc.tensor.matmul(op_t[:], v_sb[:, ts(b, 128)], pt[:],
                                     start=True, stop=True)
                    if cfg["t2_oc_engine"] == "act":
                        nc.scalar.copy(ot[:, l0:l0 + LBLK], op_t[:])
                    else:
                        nc.vector.tensor_copy(ot[:, l0:l0 + LBLK], op_t[:])
                rsx = rsp.tile([NBLK, LBLK], f32, tag="rsx")
                if cfg["t2_rs_copy"] == "act":
                    nc.scalar.copy(rsx[:], rs16[:])
                else:
                    nc.vector.tensor_copy(rsx[:], rs16[:])
                nc.sync.dma_start(rs_d[b], rsx[:])
                HO = L // NOD
                for h in range(NOD):
                    nc.sync.dma_start(outT_d[b, :, h * HO:(h + 1) * HO],
                                      ot[:, h * HO:(h + 1) * HO])

            def body(_iv=None):
                for b in range(B_LOC):
                    t2_batch(b)

            if repeat == 1:
                body()
            else:
                with tc.For_i(0, repeat, 1) as _i:
                    body(_i)

    nc.compile()
    return nc


def _build_nc(repeat=1, cfg=None):
    cfg = {**CFG, **(cfg or {})}
    if cfg["dataflow"] == "t2":
        return _build_nc_t2(repeat, cfg)
    import concourse.bacc as bacc
    import concourse.mybir as mybir
    import concourse.tile as tile
    from concourse.bass import ts
    from concourse.masks import make_identity

    f32 = mybir.dt.float32
    f32r = mybir.dt.float32r

    nc = bacc.Bacc("TRN2", target_bir_lowering=False, debug=False)
    bf16 = mybir.dt.bfloat16
    if cfg["mm1_dtype"] == "bf16x2":
        qT_d = nc.dram_tensor("qT2", (B_LOC, S, 2, L), bf16, kind="ExternalInput")
        kph_d = nc.dram_tensor("kph", (B_LOC, S, S), bf16, kind="ExternalInput")
        kpl_d = nc.dram_tensor("kpl", (B_LOC, S, S), bf16, kind="ExternalInput")
    else:
        mm1_dt_glob = f32 if cfg["mm1_dtype"] == "f32" else f32r
        qT_d = nc.dram_tensor("qT", (B_LOC, S, L), mm1_dt_glob, kind="ExternalInput")
    kp_d = nc.dram_tensor("kp", (B_LOC, S, S), f32, kind="ExternalInput")
    v_d = nc.dram_tensor("v", (B_LOC, S, S), f32, kind="ExternalInput")
    ind_d = None
    if cfg["bias_mode"] == "pe":
        ind_d = nc.dram_tensor("ind", (NT, LBLK), f32r, kind="ExternalInput")
    sel16_d = None
    if cfg["dataflow"] == "t" or cfg["rowsum_mode"] == "pe_pt":
        sel16_d = nc.dram_tensor("sel16", (S, NBLK * NBLK), f32r,
                                 kind="ExternalInput")
    out_dt = f32 if cfg["out_dtype"] == "f32" else mybir.dt.bfloat16
    outT_d = nc.dram_tensor("outT", (B_LOC, S, L), out_dt, kind="ExternalOutput")
    if cfg["rowsum_mode"] == "dve" and cfg["dataflow"] == "lsoft":
        rs_d = nc.dram_tensor("rs", (B_LOC, S, L // S), f32, kind="ExternalOutput")
    elif cfg["dataflow"] == "t" or cfg["rowsum_mode"] == "pe_pt":
        rs_d = nc.dram_tensor("rs", (B_LOC, NBLK, LBLK), f32, kind="ExternalOutput")
    else:
        rs_d = nc.dram_tensor("rs", (B_LOC, L), f32, kind="ExternalOutput")

    from concourse import bass_isa
    Exp = mybir.ActivationFunctionType.Exp
    AX = mybir.AxisListType.X
    MAX = mybir.AluOpType.max
    ADD = mybir.AluOpType.add

    with tile.TileContext(nc) as tc:
        with (
            tc.tile_pool(name="const", bufs=1) as constp,
            tc.tile_pool(name="qt", bufs=cfg["qt_bufs"]) as qtp,
            tc.tile_pool(name="p", bufs=cfg["p_bufs"]) as pp,
            tc.tile_pool(name="pt", bufs=cfg["pt_bufs"]) as ptp,
            tc.tile_pool(name="nm", bufs=cfg["nm_bufs"]) as nmp,
            tc.tile_pool(name="rss", bufs=cfg["rs_bufs"]) as rsp,
            tc.tile_pool(name="oc", bufs=cfg["oc_bufs"]) as ocp,
            tc.tile_pool(name="scps", bufs=cfg["sc_bufs"], space="PSUM") as scps,
            tc.tile_pool(name="ptps", bufs=cfg["ptps_bufs"], space="PSUM") as ptps,
            tc.tile_pool(name="ops", bufs=cfg["o_bufs"], space="PSUM") as ops,
            tc.tile_pool(name="rsps", bufs=cfg["rsps_bufs"], space="PSUM") as rspsp,
            tc.tile_pool(name="auxps", bufs=1, space="PSUM") as auxps,
        ):
            ident = constp.tile([128, 128], f32, tag="ident")
            make_identity(nc, ident[:])
            kp_sb = constp.tile([128, B_LOC * 128], f32, tag="kp")
            v_sb = constp.tile([128, B_LOC * 128], f32, tag="v")
            v_r = constp.tile([128, B_LOC * 128], f32r, tag="vr")
            ind_r = None
            if cfg["bias_mode"] == "pe":
                ind_r = constp.tile([NT, LBLK], f32r, tag="ind")
                nc.sync.dma_start(ind_r[:], ind_d[:])
            ones_r = None
            if cfg["rowsum_mode"] == "pe" or cfg["dataflow"] == "t":
                ones_f = constp.tile([128, 1], f32, tag="ones_f")
                ones_r = constp.tile([128, 1], f32r, tag="ones")
                nc.gpsimd.memset(ones_f[:], 1.0)
                nc.vector.tensor_copy(ones_r[:], ones_f[:])
            neg_inv_r = None
            sel16_r = None
            if cfg["rowsum_mode"] == "pe_pt" and cfg["dataflow"] != "t":
                sel16_r = constp.tile([128, NBLK * NBLK], f32r, tag="sel16")
                nc.sync.dma_start(sel16_r[:], sel16_d[:])
            if cfg["dataflow"] == "t":
                neg_inv_f = constp.tile([128, 128], f32, tag="ninv_f")
                neg_inv_r = constp.tile([128, 128], f32r, tag="ninv")
                nc.gpsimd.memset(neg_inv_f[:], -1.0 / 128.0)
                nc.vector.tensor_copy(neg_inv_r[:], neg_inv_f[:])
                sel16_r = constp.tile([128, NBLK * NBLK], f32r, tag="sel16")
                nc.sync.dma_start(sel16_r[:], sel16_d[:])
            for b in range(B_LOC):
                nc.sync.dma_start(kp_sb[:, ts(b, 128)], kp_d[b])
                nc.sync.dma_start(v_sb[:, ts(b, 128)], v_d[b])
            nc.vector.tensor_copy(v_r[:], v_sb[:])
            kp_r = None
            if cfg["mm1_dtype"] == "f32r":
                kp_r = constp.tile([128, B_LOC * 128], f32r, tag="kpr")
                nc.vector.tensor_copy(kp_r[:], kp_sb[:])
            kph_sb = kpl_sb = None
            if cfg["mm1_dtype"] == "bf16x2":
                bf16_ = mybir.dt.bfloat16
                kph_sb = constp.tile([128, B_LOC * 128], bf16_, tag="kph")
                kpl_sb = constp.tile([128, B_LOC * 128], bf16_, tag="kpl")
                for b in range(B_LOC):
                    nc.sync.dma_start(kph_sb[:, ts(b, 128)], kph_d[b])
                    nc.sync.dma_start(kpl_sb[:, ts(b, 128)], kpl_d[b])

            def t_block(b, blk, rs_stage, rs_ps_holder):
                l0 = blk * LBLK
                sc = scps.tile([128, LBLK], f32, tag="sc")
                if cfg["mm1_dtype"] == "bf16x2":
                    bf16_ = mybir.dt.bfloat16
                    qt2 = qtp.tile([128, 2 * LBLK], bf16_, tag="qt")
                    nc.sync.dma_start(
                        qt2[:].rearrange("p (h l) -> p h l", h=2),
                        qT_d[b, :, :, l0:l0 + LBLK])
                    qh = qt2[:, 0:LBLK]
                    ql = qt2[:, LBLK:2 * LBLK]
                    nc.tensor.matmul(sc[:], kph_sb[:, ts(b, 128)], qh,
                                     start=True, stop=False)
                    nc.tensor.matmul(sc[:], kpl_sb[:, ts(b, 128)], qh,
                                     start=False, stop=False)
                    nc.tensor.matmul(sc[:], kph_sb[:, ts(b, 128)], ql,
                                     start=False, stop=False)
                else:
                    mm1_dt = f32 if cfg["mm1_dtype"] == "f32" else f32r
                    kp_use = kp_sb if cfg["mm1_dtype"] == "f32" else kp_r
                    qt = qtp.tile([128, LBLK], mm1_dt, tag="qt")
                    nc.sync.dma_start(qt[:], qT_d[b, :, l0:l0 + LBLK])
                    nc.tensor.matmul(sc[:], kp_use[:, ts(b, 128)], qt[:],
                                     start=True, stop=False)
                scs = pp.tile([128, LBLK], f32, tag="scs")
                if cfg["scs_copy_engine"] == "dve":
                    nc.vector.tensor_copy(scs[:], sc[:])
                else:
                    nc.scalar.copy(scs[:], sc[:])
                mx_dt = f32r if cfg["max_out_dtype"] == "f32r" else f32
                mxr = ptp.tile([128, LBLK], mx_dt, tag="mxr")
                nc.gpsimd.partition_all_reduce(
                    mxr[:], scs[:], 128, bass_isa.ReduceOp.max)
                nc.tensor.matmul(sc[:], neg_inv_r[:], mxr[:],
                                 start=False, stop=True)
                pt = ptp.tile([128, LBLK], f32r, tag="pt")
                nc.scalar.activation(pt[:], sc[:], Exp, bias=0.0, scale=1.0)
                if blk == 0:
                    rs_ps_new = rspsp.tile([NBLK, LBLK], f32, tag="rsps")
                    rs_ps_holder[0] = rs_ps_new
                rs_ps = rs_ps_holder[0]
                nc.tensor.matmul(rs_ps[:], sel16_r[:, blk * NBLK:(blk + 1) * NBLK],
                                 pt[:], start=(blk == 0), stop=(blk == NBLK - 1))
                if blk == NBLK - 1:
                    rsx = rsp.tile([NBLK, LBLK], f32, tag="rsx")
                    nc.vector.tensor_copy(rsx[:], rs_ps[:])
                    nc.sync.dma_start(rs_d[b], rsx[:])
                op_t = ops.tile([128, LBLK], f32, tag="op")
                nc.tensor.matmul(op_t[:], v_r[:, ts(b, 128)], pt[:],
                                 start=True, stop=True)
                oc = ocp.tile([128, LBLK], out_dt, tag="oc")
                oce = cfg["out_copy_engine"]
                if oce in ("alt", "act") or oce.startswith("mix"):
                    nc.scalar.copy(oc[:], op_t[:])
                else:
                    nc.vector.tensor_copy(oc[:], op_t[:])
                st = {"pool": nc.gpsimd, "act": nc.scalar,
                      "dve": nc.vector, "sp": nc.sync}[cfg["store_engine"]]
                st.dma_start(outT_d[b, :, l0:l0 + LBLK], oc[:])

            def t_body(_iv=None):
                for b in range(B_LOC):
                    holder = [None]
                    for blk in range(NBLK):
                        t_block(b, blk, None, holder)

            def body(_iv=None):
                if cfg["dataflow"] == "t":
                    return t_body(_iv)
                for b in range(B_LOC):
                    mode = cfg["rowsum_mode"]
                    rs16_holder = [None]
                    rs_stage = None
                    if mode == "dve":
                        rs_stage = rsp.tile([128, L // S], f32, tag="rss")
                    elif mode == "pool":
                        rs_stage = rsp.tile([128, L], f32, tag="rss")
                    for blk in range(NBLK):
                        l0 = blk * LBLK
                        mm1_dt = f32 if cfg["mm1_dtype"] == "f32" else f32r
                        qt = qtp.tile([128, LBLK], mm1_dt, tag="qt")
                        nc.sync.dma_start(qt[:], qT_d[b, :, l0:l0 + LBLK])
                        sc = scps.tile([128, LBLK], f32, tag="sc")
                        for ti in range(NT):
                            nc.tensor.matmul(
                                sc[:, ts(ti, 128)], qt[:, ts(ti, 128)],
                                (kp_sb if cfg["mm1_dtype"] == "f32" else kp_r)[:, ts(b, 128)],
                                start=True,
                                stop=(cfg["bias_mode"] == "act"),
                                skip_group_check=(cfg["bias_mode"] == "pe"))
                        nm = nmp.tile([128, NT], f32, tag="nm")
                        nc.vector.tensor_reduce(
                            nm[:], sc[:].rearrange("p (t s) -> p t s", t=NT),
                            axis=AX, op=MAX, negate=True)
                        p = pp.tile([128, LBLK], f32, tag="p")
                        if cfg["bias_mode"] == "act":
                            for ti in range(NT):
                                nc.scalar.activation(
                                    p[:, ts(ti, 128)], sc[:, ts(ti, 128)], Exp,
                                    bias=nm[:, ti:ti + 1], scale=1.0)
                        else:
                            nmt_ps = auxps.tile([NT, 128], f32, tag="nmt")
                            nc.tensor.transpose(nmt_ps[:], nm[:], ident[:])
                            nmt = nmp.tile([NT, 128], f32r, tag="nmtr")
                            if cfg["nm_copy_engine"] == "dve":
                                nc.vector.tensor_copy(nmt[:], nmt_ps[:])
                            else:
                                nc.scalar.copy(nmt[:], nmt_ps[:])
                            nc.tensor.matmul(sc[:], nmt[:], ind_r[:],
                                             start=False, stop=True,
                                             skip_group_check=True)
                            nc.scalar.activation(p[:], sc[:], Exp,
                                                 bias=0.0, scale=1.0)
                        if cfg["rowsum_mode"] == "dve":
                            nc.vector.tensor_reduce(
                                rs_stage[:, blk * NT:(blk + 1) * NT],
                                p[:].rearrange("p (t s) -> p t s", t=NT),
                                axis=AX, op=ADD)
                        ptps_t = ptps.tile([128, LBLK], f32, tag="ptps")
                        for ti in range(NT):
                            nc.tensor.transpose(
                                ptps_t[:, ts(ti, 128)], p[:, ts(ti, 128)],
                                ident[:])
                        pt = ptp.tile([128, LBLK], f32r, tag="pt")
                        pce = cfg["pt_copy_engine"]
                        if pce == "alt":
                            pce = "dve" if blk % 2 == 0 else "act"
                        elif pce.startswith("mix"):
                            n, m = pce[3:].split("of")
                            pce = "dve" if blk % int(m) < int(n) else "act"
                        if pce == "dve":
                            nc.vector.tensor_copy(pt[:], ptps_t[:])
                        else:
                            nc.scalar.copy(pt[:], ptps_t[:])
                        if cfg["rowsum_mode"] == "pool":
                            nc.gpsimd.partition_all_reduce(
                                rs_stage[:, blk * LBLK:(blk + 1) * LBLK],
                                pt[:], 128, bass_isa.ReduceOp.add)
                        elif cfg["rowsum_mode"] == "pe":
                            if blk % 4 == 0:
                                rs_ps = rspsp.tile([128, LBLK], f32, tag="rsps")
                            j = blk % 4
                            nc.tensor.matmul(
                                rs_ps[32 * j:32 * j + 1, :], ones_r[:], pt[:],
                                start=True, stop=True,
                                tile_position=(0, 32 * j))
                            if j == 3:
                                nc.vector.tensor_copy(
                                    rs_stage[(blk - 3) // 4 * 4:(blk - 3) // 4 * 4 + 4, :].rearrange("a b -> a b"),
                                    rs_ps[:].rearrange("(a c) b -> a c b", c=32)[:, 0:1, :].rearrange("a c b -> (a c) b"))
                        if cfg["rowsum_mode"] == "pe_pt":
                            if blk == 0:
                                rs16_new = rspsp.tile([NBLK, LBLK], f32,
                                                      tag="rsps")
                                rs16_holder[0] = rs16_new
                            rs16 = rs16_holder[0]
                            nc.tensor.matmul(
                                rs16[:],
                                sel16_r[:, blk * NBLK:(blk + 1) * NBLK],
                                pt[:], start=(blk == 0),
                                stop=(blk == NBLK - 1))
                            if blk == NBLK - 1:
                                rsx = rsp.tile([NBLK, LBLK], f32, tag="rsx")
                                nc.vector.tensor_copy(rsx[:], rs16[:])
                                nc.sync.dma_start(rs_d[b], rsx[:])
                        op_t = ops.tile([128, LBLK], f32, tag="op")
                        nc.tensor.matmul(
                            op_t[:], v_r[:, ts(b, 128)], pt[:],
                            start=True, stop=True)
                        oc = ocp.tile([128, LBLK], out_dt, tag="oc")
                        oce = cfg["out_copy_engine"]
                        if oce == "alt":
                            oce = "act" if blk % 2 == 0 else "dve"
                        elif oce.startswith("mix"):
                            n, m = oce[3:].split("of")
                            oce = "dve" if blk % int(m) < int(n) else "act"
                        if oce == "act":
                            nc.scalar.copy(oc[:], op_t[:])
                        else:
                            nc.vector.tensor_copy(oc[:], op_t[:])
                        st = {"pool": nc.gpsimd, "act": nc.scalar,
                              "dve": nc.vector, "sp": nc.sync}[cfg["store_engine"]]
                        st.dma_start(outT_d[b, :, l0:l0 + LBLK], oc[:])
                    if cfg["rowsum_mode"] == "dve":
                        nc.gpsimd.dma_start(rs_d[b], rs_stage[:])
                    elif cfg["rowsum_mode"] == "pool":
                        nc.sync.dma_start(rs_d[b], rs_stage[0:1, :].rearrange("a b -> (a b)"))

            if repeat == 1:
                body()
            else:
                with tc.For_i(0, repeat, 1) as _i:
                    body(_i)

    nc.compile()
    return nc


def _make_runner(repeat=1, cfg=None):
    """Compile (once) and return fn(in_maps) -> list[dict] per core."""
    key = (repeat, tuple(sorted((cfg or {}).items())))
    if key in _RUNNER_CACHE:
        return _RUNNER_CACHE[key]

    import jax
    import concourse.mybir as mybir
    from concourse import bass2jax
    from concourse.bass2jax import _bass_exec_p, partition_id_tensor
    from jax.sharding import Mesh, NamedSharding, PartitionSpec
    from jax.experimental.shard_map import shard_map

    nc = _build_nc(repeat, cfg)
    bass2jax.install_neuronx_cc_hook()

    in_names, out_names, out_avals, zero_shapes = [], [], [], []
    for alloc in nc.m.functions[0].allocations:
        if not isinstance(alloc, mybir.MemoryLocationSet):
            continue
        name = alloc.memorylocations[0].name
        if alloc.kind == "ExternalInput":
            if nc.partition_id_tensor is None or name != nc.partition_id_tensor.name:
                in_names.append(name)
        elif alloc.kind == "ExternalOutput":
            out_names.append(name)
            shape = tuple(alloc.tensor_shape)
            dtype = mybir.dt.np(alloc.dtype)
            out_avals.append(jax.core.ShapedArray(shape, dtype))
            zero_shapes.append((shape, dtype))
    n_params = len(in_names)
    pid_name = nc.partition_id_tensor.name if nc.partition_id_tensor else None
    names_for_bind = in_names + out_names + ([pid_name] if pid_name else [])

    def _body(*args):
        operands = list(args)
        if pid_name:
            operands.append(partition_id_tensor())
        outs = _bass_exec_p.bind(
            *operands,
            out_avals=tuple(out_avals),
            in_names=tuple(names_for_bind),
            out_names=tuple(out_names),
            lowering_input_output_aliases=(),
            sim_require_finite=True,
            sim_require_nnan=True,
            nc=nc,
        )
        return tuple(outs)

    devices = jax.devices()[:N_CORES]
    mesh = Mesh(np.asarray(devices), ("core",))
    nspec = n_params + len(out_names)
    fn = jax.jit(
        shard_map(_body, mesh=mesh,
                  in_specs=(PartitionSpec("core"),) * nspec,
                  out_specs=(PartitionSpec("core"),) * len(out_names),
                  check_rep=False),
        keep_unused=True)
    sharding = NamedSharding(mesh, PartitionSpec("core"))

    def run(in_maps):
        import jax as _jax
        concat_in = [
            np.concatenate([np.asarray(m[name]) for m in in_maps], axis=0)
            for name in in_names
        ]
        zeros = [np.zeros((N_CORES * s[0],) + tuple(s[1:]), d)
                 for (s, d) in zero_shapes]
        dev_in = [_jax.device_put(a, sharding) for a in concat_in + zeros]
        out_arrs = fn(*dev_in)
        _jax.block_until_ready(out_arrs)
        return [
            {name: np.asarray(out_arrs[i]).reshape(
                (N_CORES,) + tuple(out_avals[i].shape))[c]
             for i, name in enumerate(out_names)}
            for c in range(N_CORES)
        ], (fn, dev_in)

    _RUNNER_CACHE[key] = run
    return run


def _prep_inputs_t2(queries, keys, values):
    import ml_dtypes
    bf = ml_dtypes.bfloat16
    q = np.asarray(queries, np.float32)
    k = np.asarray(keys, np.float32)
    v = np.asarray(values, np.float32)
    qT16 = np.ascontiguousarray(q.transpose(0, 2, 1).astype(np.float16))
    k16 = np.ascontiguousarray(k.astype(np.float16))           # (B, e, j)
    v16 = np.ascontiguousarray(v.astype(bf))                   # (B, j, d)
    qsum = q.sum(axis=2)                                       # (B, L) f32
    krow = k.sum(axis=2)                                       # (B, S) f32
    f16 = np.float16
    qh = qsum.astype(f16)
    ql = (qsum - qh.astype(np.float32)).astype(f16)
    kh = krow.astype(f16)
    kl = (krow - kh.astype(np.float32)).astype(f16)
    c = np.maximum(qsum * krow.max(axis=1)[:, None],
                   qsum * krow.min(axis=1)[:, None])           # (B, L)
    cb = c.astype(f16)
    ones = np.ones((B, S), np.float32).astype(f16)
    alhs = np.ascontiguousarray(
        np.stack([kh, kl, kh, -ones], axis=1))                 # (B, 4, S)
    arhs = np.ascontiguousarray(
        np.stack([qh, qh, ql, cb], axis=1))                    # (B, 4, L)
    sel = np.zeros((S, NBLK * NBLK), np.float32)
    for j in range(NBLK):
        sel[:, j * NBLK + j] = 1.0
    selb = sel.astype(bf)
    in_maps = []
    for ci in range(N_CORES):
        sl = slice(ci * B_LOC, (ci + 1) * B_LOC)
        in_maps.append({"qT16": qT16[sl], "k16": k16[sl], "v16": v16[sl],
                        "alhs": alhs[sl], "arhs": arhs[sl], "selb": selb})
    return in_maps


def _assemble_t2(results):
    out = np.empty((B, L, S), dtype=np.float32)
    for ci in range(N_CORES):
        outT = results[ci]["outT"]         # (B_LOC, S, L) bf16, = out^T
        rs = results[ci]["rs"]             # (B_LOC, NBLK, LBLK) f32
        for b in range(B_LOC):
            rsum = rs[b].reshape(L)
            out[ci * B_LOC + b] = outT[b].astype(np.float32).T / rsum[:, None]
    return out.reshape(B, 1, L, S)


def _prep_inputs(queries, keys, values, cfg=None):
    cfg = {**CFG, **(cfg or {})}
    if cfg["dataflow"] == "t2":
        return _prep_inputs_t2(queries, keys, values)
    qT = np.ascontiguousarray(queries.transpose(0, 2, 1))      # (B, E, L)
    kp = keys + keys.sum(axis=2)[:, None, :]                   # k' = k + 1*ksum
    kp = np.ascontiguousarray(kp.astype(np.float32))
    v = np.ascontiguousarray(values.astype(np.float32))
    qT2 = kph = kpl = None
    if cfg["mm1_dtype"] == "bf16x2":
        import ml_dtypes
        bf = ml_dtypes.bfloat16
        qTh = qT.astype(bf)
        qTl = (qT - qTh.astype(np.float32)).astype(bf)
        qT2 = np.ascontiguousarray(np.stack([qTh, qTl], axis=2))
        kph = kp.astype(bf)
        kpl = np.ascontiguousarray((kp - kph.astype(np.float32)).astype(bf))
        kph = np.ascontiguousarray(kph)
    ind = np.zeros((NT, LBLK), np.float32)
    for ti in range(NT):
        ind[ti, ti * 128:(ti + 1) * 128] = 1.0
    sel16 = np.zeros((S, NBLK * NBLK), np.float32)
    for j in range(NBLK):
        sel16[:, j * NBLK + j] = 1.0
    in_maps = []
    for c in range(N_CORES):
        sl = slice(c * B_LOC, (c + 1) * B_LOC)
        m = {"qT": qT[sl], "kp": kp[sl], "v": v[sl], "ind": ind,
             "sel16": sel16}
        if qT2 is not None:
            m.update({"qT2": qT2[sl], "kph": kph[sl], "kpl": kpl[sl]})
        in_maps.append(m)
    return in_maps


def _assemble(results, cfg=None):
    cfg = {**CFG, **(cfg or {})}
    if cfg["dataflow"] == "t2":
        return _assemble_t2(results)
    out = np.empty((B, L, S), dtype=np.float32)
    for c in range(N_CORES):
        outT = results[c]["outT"]          # (B_LOC, S, L)  = out^T per batch
        rs = results[c]["rs"]              # (B_LOC, 128, L//128) rowsums
        for b in range(B_LOC):
            if cfg["rowsum_mode"] == "dve" and cfg["dataflow"] == "lsoft":
                rsum = rs[b].T.reshape(L)  # rowsum[l]
            else:
                rsum = rs[b].reshape(L)
            out[c * B_LOC + b] = outT[b].T / rsum[:, None]
    return out.reshape(B, 1, L, S)


def kernel(queries, keys, values):
    run = _make_runner(repeat=1)
    in_maps = _prep_inputs(queries, keys, values)
    results, _ = run(in_maps)
    return _assemble(results)



# revision 15
# speedup vs baseline: 1.2191x; 1.0245x over previous
"""Trainium2 Bass kernel for an AxialAttentionLayer-style module.

Math: for each batch b,
    scores = q @ k'          where k'[e,j] = keys[e,j] + sum_d keys[j,d]
    A      = softmax(scores, axis=-1)
    out    = A @ values
(the reference's rank-1 additive score s1 folds into the matmul because
 s1[l,j] = (sum_e q[l,e]) * ksum[j] = sum_e q[l,e]*ksum[j]).

Sharding: data-parallel over batch B=32 across 8 cores (4 batches/core).
Device layout per core, per 512-row block of L:
    MM1   (PE, fp32):   scores(l,s) = qT_chunk.T @ k'   (4x 128-tiles)
    max   (DVE):        fused 3D reduce_max(negate) -> -rowmax (128,4)
    exp   (ACT):        P = exp(scores - rowmax), per-tile bias
    rowsum(DVE):        fused 3D reduce_sum over P -> staged per batch
    T     (PE):         P chunks transposed via PE transpose -> PSUM
    copy  (ACT):        PT PSUM -> SBUF (rounded to fp32r)
    MM2   (PE, fp32r):  outT(d,l) = v.T-free matmul with v stationary
    copy  (DVE):        outT PSUM -> SBUF
Host: pre-transposes q -> qT, builds k', divides by rowsum, transposes back.
"""

import numpy as np

B, L, S = 32, 8192, 128
N_CORES = 8
B_LOC = B // N_CORES  # 4
LBLK = 512            # l-rows per block
NT = LBLK // 128      # 128-tiles per block
NBLK = L // LBLK      # blocks per batch

_RUNNER_CACHE = {}

# tunables (overridable before building)
CFG = dict(
    qt_bufs=8, p_bufs=4, pt_bufs=4, nm_bufs=4, oc_bufs=6, rs_bufs=2,
    sc_bufs=4, ptps_bufs=2, o_bufs=2,
    out_copy_engine="dve",   # "act" | "dve" | "alt"
    pt_copy_engine="act",    # "act" | "dve" | "alt"
    store_engine="sp",       # "pool" | "act" | "dve" | "sp"
    rowsum_mode="dve",       # "dve" | "pool" | "pe"
    mm1_dtype="f32",         # "f32" | "f32r" | "bf16x2"
    out_dtype="f32",         # "f32" | "bf16"
    bias_mode="act",         # "act" (per-tile exp bias) | "pe" (K=4 accum matmul)
    nm_copy_engine="act",    # "dve" | "act"
    scs_copy_engine="dve",   # "dve" | "act"
    dataflow="t2",           # "lsoft" | "t" | "t2" (fp16 MM1 + host shift)
    rsps_bufs=1,
    max_out_dtype="f32r",    # partition_all_reduce out dtype in "t" flow
    # --- t2 knobs ---
    t2_rowsum="pe",          # "pe" (sel16 matmul) | "pool" (gpsimd all-reduce)
    t2_oc_engine="dve",      # "dve" | "act"
    t2_rs_copy="dve",        # "dve" | "act" | "pool"
    t2_sc_bufs=3, t2_o_bufs=2, t2_rsps_bufs=2,
    t2_qs_bufs=2, t2_ot_bufs=2, t2_q_dmas=2, t2_o_dmas=1,
    t2_mm_dtype="f16",       # "f16" | "bf16" (bf16: timing diagnostics only)
    t2_skip_rowsum=False,    # True: omit rowsum matmul+DMA (diagnostics)
    t2_skip_additive=False,  # True: omit additive matmul (diagnostics)
    t2_swp=True,             # software-pipeline PE stream across blocks
)


def _build_nc_t2(repeat=1, cfg=None):
    """fp16 MM1 + K=4 bf16 additive (rank-1 score + host softmax shift),
    no device max, bf16 P/V/out. Layout: scores in (j part, l free)."""
    cfg = {**CFG, **(cfg or {})}
    import concourse.bacc as bacc
    import concourse.mybir as mybir
    import concourse.tile as tile
    from concourse.bass import ts
    from concourse import bass_isa

    f32 = mybir.dt.float32
    f16 = mybir.dt.float16 if cfg["t2_mm_dtype"] == "f16" else mybir.dt.bfloat16
    bf16 = mybir.dt.bfloat16
    Exp = mybir.ActivationFunctionType.Exp

    nc = bacc.Bacc("TRN2", target_bir_lowering=False, debug=False)
    qT_d = nc.dram_tensor("qT16", (B_LOC, S, L), f16, kind="ExternalInput")
    k_d = nc.dram_tensor("k16", (B_LOC, S, S), f16, kind="ExternalInput")
    v_d = nc.dram_tensor("v16", (B_LOC, S, S), bf16, kind="ExternalInput")
    alhs_d = nc.dram_tensor("alhs", (B_LOC, 4, S), f16, kind="ExternalInput")
    arhs_d = nc.dram_tensor("arhs", (B_LOC, 4, L), f16, kind="ExternalInput")
    sel_d = nc.dram_tensor("selb", (S, NBLK * NBLK), bf16, kind="ExternalInput")
    outT_d = nc.dram_tensor("outT", (B_LOC, S, L), bf16, kind="ExternalOutput")
    rs_d = nc.dram_tensor("rs", (B_LOC, NBLK, LBLK), f32, kind="ExternalOutput")

    NQD = cfg["t2_q_dmas"]     # DMAs per batch for q load
    NOD = cfg["t2_o_dmas"]     # DMAs per batch for out store
    with tile.TileContext(nc) as tc:
        with (
            tc.tile_pool(name="const", bufs=1) as constp,
            tc.tile_pool(name="qs", bufs=cfg["t2_qs_bufs"]) as qsp,
            tc.tile_pool(name="ot", bufs=cfg["t2_ot_bufs"]) as otp,
            tc.tile_pool(name="pt", bufs=cfg["pt_bufs"]) as ptp,
            tc.tile_pool(name="rss", bufs=cfg["rs_bufs"]) as rsp,
            tc.tile_pool(name="scps", bufs=cfg["t2_sc_bufs"], space="PSUM") as scps,
            tc.tile_pool(name="ops", bufs=cfg["t2_o_bufs"], space="PSUM") as ops,
            tc.tile_pool(name="rsps", bufs=cfg["t2_rsps_bufs"], space="PSUM") as rspsp,
        ):
            k_sb = constp.tile([128, B_LOC * 128], f16, tag="k16")
            v_sb = constp.tile([128, B_LOC * 128], bf16, tag="v16")
            alhs_sb = constp.tile([4, B_LOC * 128], f16, tag="alhs")
            arc_sb = constp.tile([4, B_LOC * L], f16, tag="arc")
            sel_sb = constp.tile([128, NBLK * NBLK], bf16, tag="selb")
            nc.sync.dma_start(sel_sb[:], sel_d[:])
            for b in range(B_LOC):
                nc.sync.dma_start(k_sb[:, ts(b, 128)], k_d[b])
                nc.sync.dma_start(v_sb[:, ts(b, 128)], v_d[b])
                nc.sync.dma_start(alhs_sb[:, ts(b, 128)], alhs_d[b])
                nc.sync.dma_start(arc_sb[:, b * L:(b + 1) * L], arhs_d[b])

            state = {}

            def stage_mm1(b, blk):
                l0 = blk * LBLK
                sc = scps.tile([128, LBLK], f32, tag="sc")
                nc.tensor.matmul(sc[:], k_sb[:, ts(b, 128)],
                                 qs_cur[0][:, l0:l0 + LBLK],
                                 start=True,
                                 stop=cfg["t2_skip_additive"])
                if not cfg["t2_skip_additive"]:
                    nc.tensor.matmul(sc[:], alhs_sb[:, ts(b, 128)],
                                     arc_sb[:, b * L + l0:b * L + l0 + LBLK],
                                     start=False, stop=True)
                return sc

            def stage_rest(b, blk, sc):
                l0 = blk * LBLK
                pt = ptp.tile([128, LBLK], bf16, tag="pt")
                nc.scalar.activation(pt[:], sc[:], Exp, bias=0.0, scale=1.0)
                rsmode = ("skip" if cfg["t2_skip_rowsum"] else cfg["t2_rowsum"])
                if rsmode == "pe":
                    if blk == 0:
                        rs16_new = rspsp.tile([NBLK, LBLK], f32, tag="rsps")
                        state["rs16"] = rs16_new
                    nc.tensor.matmul(
                        state["rs16"][:],
                        sel_sb[:, blk * NBLK:(blk + 1) * NBLK],
                        pt[:], start=(blk == 0), stop=(blk == NBLK - 1))
                elif rsmode == "pool":
                    prt = rsp.tile([128, LBLK], f32, tag="prt")
                    nc.gpsimd.partition_all_reduce(
                        prt[:], pt[:], 128, bass_isa.ReduceOp.add)
                    nc.gpsimd.dma_start(
                        rs_d[b, blk], prt[0:1, :].rearrange("a b -> (a b)"))
                op_t = ops.tile([128, LBLK], f32, tag="op")
                nc.tensor.matmul(op_t[:], v_sb[:, ts(b, 128)], pt[:],
                                 start=True, stop=True)
                if cfg["t2_oc_engine"] == "act":
                    nc.scalar.copy(ot_cur[0][:, l0:l0 + LBLK], op_t[:])
                else:
                    nc.vector.tensor_copy(ot_cur[0][:, l0:l0 + LBLK], op_t[:])
                if blk == NBLK - 1:
                    if rsmode == "pe":
                        rsx = rsp.tile([NBLK, LBLK], f32, tag="rsx")
                        if cfg["t2_rs_copy"] == "act":
                            nc.scalar.copy(rsx[:], state["rs16"][:])
                        else:
                            nc.vector.tensor_copy(rsx[:], state["rs16"][:])
                        nc.sync.dma_start(rs_d[b], rsx[:])
                    HO = L // NOD
                    for h in range(NOD):
                        nc.sync.dma_start(
                            outT_d[b, :, h * HO:(h + 1) * HO],
                            ot_cur[0][:, h * HO:(h + 1) * HO])

            qs_cur = [None]
            ot_cur = [None]

            def load_batch(b):
                qs = qsp.tile([128, L], f16, tag="qs")
                H = L // NQD
                for h in range(NQD):
                    nc.sync.dma_start(qs[:, h * H:(h + 1) * H],
                                      qT_d[b, :, h * H:(h + 1) * H])
                return qs

            def body(_iv=None):
                if not cfg["t2_swp"]:
                    for b in range(B_LOC):
                        qs_cur[0] = load_batch(b)
                        ot_new = otp.tile([128, L], bf16, tag="ot")
                        ot_cur[0] = ot_new
                        for blk in range(NBLK):
                            sc = stage_mm1(b, blk)
                            stage_rest(b, blk, sc)
                    return
                # software-pipelined: PE runs mm1(i+1) before rs/mm2(i)
                items = [(b, blk) for b in range(B_LOC)
                         for blk in range(NBLK)]
                pend = None
                for i, (b, blk) in enumerate(items):
                    if blk == 0:
                        if b == 0:
                            qs_cur[0] = load_batch(b)
                        state[("qs_next", b)] = None
                        ot_new = otp.tile([128, L], bf16, tag="ot")
                        state[("ot", b)] = ot_new
                    if blk == 1 and b + 1 < B_LOC:
                        state[("qs_next", b)] = load_batch(b + 1)
                    sc = stage_mm1(b, blk)
                    if pend is not None:
                        pb, pblk, psc, pqs, pot = pend
                        qs_sav, ot_sav = qs_cur[0], ot_cur[0]
                        qs_cur[0], ot_cur[0] = pqs, pot
                        stage_rest(pb, pblk, psc)
                        qs_cur[0], ot_cur[0] = qs_sav, ot_sav
                    pend = (b, blk, sc, qs_cur[0], state[("ot", b)])
                    if blk == NBLK - 1 and b + 1 < B_LOC:
                        qs_cur[0] = state[("qs_next", b)]
                pb, pblk, psc, pqs, pot = pend
                qs_cur[0], ot_cur[0] = pqs, pot
                stage_rest(pb, pblk, psc)

            if repeat == 1:
                body()
            else:
                with tc.For_i(0, repeat, 1) as _i:
                    body(_i)

    nc.compile()
    return nc


def _build_nc(repeat=1, cfg=None):
    cfg = {**CFG, **(cfg or {})}
    if cfg["dataflow"] == "t2":
        return _build_nc_t2(repeat, cfg)
    import concourse.bacc as bacc
    import concourse.mybir as mybir
    import concourse.tile as tile
    from concourse.bass import ts
    from concourse.masks import make_identity

    f32 = mybir.dt.float32
    f32r = mybir.dt.float32r

    nc = bacc.Bacc("TRN2", target_bir_lowering=False, debug=False)
    bf16 = mybir.dt.bfloat16
    if cfg["mm1_dtype"] == "bf16x2":
        qT_d = nc.dram_tensor("qT2", (B_LOC, S, 2, L), bf16, kind="ExternalInput")
        kph_d = nc.dram_tensor("kph", (B_LOC, S, S), bf16, kind="ExternalInput")
        kpl_d = nc.dram_tensor("kpl", (B_LOC, S, S), bf16, kind="ExternalInput")
    else:
        mm1_dt_glob = f32 if cfg["mm1_dtype"] == "f32" else f32r
        qT_d = nc.dram_tensor("qT", (B_LOC, S, L), mm1_dt_glob, kind="ExternalInput")
    kp_d = nc.dram_tensor("kp", (B_LOC, S, S), f32, kind="ExternalInput")
    v_d = nc.dram_tensor("v", (B_LOC, S, S), f32, kind="ExternalInput")
    ind_d = None
    if cfg["bias_mode"] == "pe":
        ind_d = nc.dram_tensor("ind", (NT, LBLK), f32r, kind="ExternalInput")
    sel16_d = None
    if cfg["dataflow"] == "t" or cfg["rowsum_mode"] == "pe_pt":
        sel16_d = nc.dram_tensor("sel16", (S, NBLK * NBLK), f32r,
                                 kind="ExternalInput")
    out_dt = f32 if cfg["out_dtype"] == "f32" else mybir.dt.bfloat16
    outT_d = nc.dram_tensor("outT", (B_LOC, S, L), out_dt, kind="ExternalOutput")
    if cfg["rowsum_mode"] == "dve" and cfg["dataflow"] == "lsoft":
        rs_d = nc.dram_tensor("rs", (B_LOC, S, L // S), f32, kind="ExternalOutput")
    elif cfg["dataflow"] == "t" or cfg["rowsum_mode"] == "pe_pt":
        rs_d = nc.dram_tensor("rs", (B_LOC, NBLK, LBLK), f32, kind="ExternalOutput")
    else:
        rs_d = nc.dram_tensor("rs", (B_LOC, L), f32, kind="ExternalOutput")

    from concourse import bass_isa
    Exp = mybir.ActivationFunctionType.Exp
    AX = mybir.AxisListType.X
    MAX = mybir.AluOpType.max
    ADD = mybir.AluOpType.add

    with tile.TileContext(nc) as tc:
        with (
            tc.tile_pool(name="const", bufs=1) as constp,
            tc.tile_pool(name="qt", bufs=cfg["qt_bufs"]) as qtp,
            tc.tile_pool(name="p", bufs=cfg["p_bufs"]) as pp,
            tc.tile_pool(name="pt", bufs=cfg["pt_bufs"]) as ptp,
            tc.tile_pool(name="nm", bufs=cfg["nm_bufs"]) as nmp,
            tc.tile_pool(name="rss", bufs=cfg["rs_bufs"]) as rsp,
            tc.tile_pool(name="oc", bufs=cfg["oc_bufs"]) as ocp,
            tc.tile_pool(name="scps", bufs=cfg["sc_bufs"], space="PSUM") as scps,
            tc.tile_pool(name="ptps", bufs=cfg["ptps_bufs"], space="PSUM") as ptps,
            tc.tile_pool(name="ops", bufs=cfg["o_bufs"], space="PSUM") as ops,
            tc.tile_pool(name="rsps", bufs=cfg["rsps_bufs"], space="PSUM") as rspsp,
            tc.tile_pool(name="auxps", bufs=1, space="PSUM") as auxps,
        ):
            ident = constp.tile([128, 128], f32, tag="ident")
            make_identity(nc, ident[:])
            kp_sb = constp.tile([128, B_LOC * 128], f32, tag="kp")
            v_sb = constp.tile([128, B_LOC * 128], f32, tag="v")
            v_r = constp.tile([128, B_LOC * 128], f32r, tag="vr")
            ind_r = None
            if cfg["bias_mode"] == "pe":
                ind_r = constp.tile([NT, LBLK], f32r, tag="ind")
                nc.sync.dma_start(ind_r[:], ind_d[:])
            ones_r = None
            if cfg["rowsum_mode"] == "pe" or cfg["dataflow"] == "t":
                ones_f = constp.tile([128, 1], f32, tag="ones_f")
                ones_r = constp.tile([128, 1], f32r, tag="ones")
                nc.gpsimd.memset(ones_f[:], 1.0)
                nc.vector.tensor_copy(ones_r[:], ones_f[:])
            neg_inv_r = None
            sel16_r = None
            if cfg["rowsum_mode"] == "pe_pt" and cfg["dataflow"] != "t":
                sel16_r = constp.tile([128, NBLK * NBLK], f32r, tag="sel16")
                nc.sync.dma_start(sel16_r[:], sel16_d[:])
            if cfg["dataflow"] == "t":
                neg_inv_f = constp.tile([128, 128], f32, tag="ninv_f")
                neg_inv_r = constp.tile([128, 128], f32r, tag="ninv")
                nc.gpsimd.memset(neg_inv_f[:], -1.0 / 128.0)
                nc.vector.tensor_copy(neg_inv_r[:], neg_inv_f[:])
                sel16_r = constp.tile([128, NBLK * NBLK], f32r, tag="sel16")
                nc.sync.dma_start(sel16_r[:], sel16_d[:])
            for b in range(B_LOC):
                nc.sync.dma_start(kp_sb[:, ts(b, 128)], kp_d[b])
                nc.sync.dma_start(v_sb[:, ts(b, 128)], v_d[b])
            nc.vector.tensor_copy(v_r[:], v_sb[:])
            kp_r = None
            if cfg["mm1_dtype"] == "f32r":
                kp_r = constp.tile([128, B_LOC * 128], f32r, tag="kpr")
                nc.vector.tensor_copy(kp_r[:], kp_sb[:])
            kph_sb = kpl_sb = None
            if cfg["mm1_dtype"] == "bf16x2":
                bf16_ = mybir.dt.bfloat16
                kph_sb = constp.tile([128, B_LOC * 128], bf16_, tag="kph")
                kpl_sb = constp.tile([128, B_LOC * 128], bf16_, tag="kpl")
                for b in range(B_LOC):
                    nc.sync.dma_start(kph_sb[:, ts(b, 128)], kph_d[b])
                    nc.sync.dma_start(kpl_sb[:, ts(b, 128)], kpl_d[b])

            def t_block(b, blk, rs_stage, rs_ps_holder):
                l0 = blk * LBLK
                sc = scps.tile([128, LBLK], f32, tag="sc")
                if cfg["mm1_dtype"] == "bf16x2":
                    bf16_ = mybir.dt.bfloat16
                    qt2 = qtp.tile([128, 2 * LBLK], bf16_, tag="qt")
                    nc.sync.dma_start(
                        qt2[:].rearrange("p (h l) -> p h l", h=2),
                        qT_d[b, :, :, l0:l0 + LBLK])
                    qh = qt2[:, 0:LBLK]
                    ql = qt2[:, LBLK:2 * LBLK]
                    nc.tensor.matmul(sc[:], kph_sb[:, ts(b, 128)], qh,
                                     start=True, stop=False)
                    nc.tensor.matmul(sc[:], kpl_sb[:, ts(b, 128)], qh,
                                     start=False, stop=False)
                    nc.tensor.matmul(sc[:], kph_sb[:, ts(b, 128)], ql,
                                     start=False, stop=False)
                else:
                    mm1_dt = f32 if cfg["mm1_dtype"] == "f32" else f32r
                    kp_use = kp_sb if cfg["mm1_dtype"] == "f32" else kp_r
                    qt = qtp.tile([128, LBLK], mm1_dt, tag="qt")
                    nc.sync.dma_start(qt[:], qT_d[b, :, l0:l0 + LBLK])
                    nc.tensor.matmul(sc[:], kp_use[:, ts(b, 128)], qt[:],
                                     start=True, stop=False)
                scs = pp.tile([128, LBLK], f32, tag="scs")
                if cfg["scs_copy_engine"] == "dve":
                    nc.vector.tensor_copy(scs[:], sc[:])
                else:
                    nc.scalar.copy(scs[:], sc[:])
                mx_dt = f32r if cfg["max_out_dtype"] == "f32r" else f32
                mxr = ptp.tile([128, LBLK], mx_dt, tag="mxr")
                nc.gpsimd.partition_all_reduce(
                    mxr[:], scs[:], 128, bass_isa.ReduceOp.max)
                nc.tensor.matmul(sc[:], neg_inv_r[:], mxr[:],
                                 start=False, stop=True)
                pt = ptp.tile([128, LBLK], f32r, tag="pt")
                nc.scalar.activation(pt[:], sc[:], Exp, bias=0.0, scale=1.0)
                if blk == 0:
                    rs_ps_new = rspsp.tile([NBLK, LBLK], f32, tag="rsps")
                    rs_ps_holder[0] = rs_ps_new
                rs_ps = rs_ps_holder[0]
                nc.tensor.matmul(rs_ps[:], sel16_r[:, blk * NBLK:(blk + 1) * NBLK],
                                 pt[:], start=(blk == 0), stop=(blk == NBLK - 1))
                if blk == NBLK - 1:
                    rsx = rsp.tile([NBLK, LBLK], f32, tag="rsx")
                    nc.vector.tensor_copy(rsx[:], rs_ps[:])
                    nc.sync.dma_start(rs_d[b], rsx[:])
                op_t = ops.tile([128, LBLK], f32, tag="op")
                nc.tensor.matmul(op_t[:], v_r[:, ts(b, 128)], pt[:],
                                 start=True, stop=True)
                oc = ocp.tile([128, LBLK], out_dt, tag="oc")
                oce = cfg["out_copy_engine"]
                if oce in ("alt", "act") or oce.startswith("mix"):
                    nc.scalar.copy(oc[:], op_t[:])
                else:
                    nc.vector.tensor_copy(oc[:], op_t[:])
                st = {"pool": nc.gpsimd, "act": nc.scalar,
                      "dve": nc.vector, "sp": nc.sync}[cfg["store_engine"]]
                st.dma_start(outT_d[b, :, l0:l0 + LBLK], oc[:])

            def t_body(_iv=None):
                for b in range(B_LOC):
                    holder = [None]
                    for blk in range(NBLK):
                        t_block(b, blk, None, holder)

            def body(_iv=None):
                if cfg["dataflow"] == "t":
                    return t_body(_iv)
                for b in range(B_LOC):
                    mode = cfg["rowsum_mode"]
                    rs16_holder = [None]
                    rs_stage = None
                    if mode == "dve":
                        rs_stage = rsp.tile([128, L // S], f32, tag="rss")
                    elif mode == "pool":
                        rs_stage = rsp.tile([128, L], f32, tag="rss")
                    for blk in range(NBLK):
                        l0 = blk * LBLK
                        mm1_dt = f32 if cfg["mm1_dtype"] == "f32" else f32r
                        qt = qtp.tile([128, LBLK], mm1_dt, tag="qt")
                        nc.sync.dma_start(qt[:], qT_d[b, :, l0:l0 + LBLK])
                        sc = scps.tile([128, LBLK], f32, tag="sc")
                        for ti in range(NT):
                            nc.tensor.matmul(
                                sc[:, ts(ti, 128)], qt[:, ts(ti, 128)],
                                (kp_sb if cfg["mm1_dtype"] == "f32" else kp_r)[:, ts(b, 128)],
                                start=True,
                                stop=(cfg["bias_mode"] == "act"),
                                skip_group_check=(cfg["bias_mode"] == "pe"))
                        nm = nmp.tile([128, NT], f32, tag="nm")
                        nc.vector.tensor_reduce(
                            nm[:], sc[:].rearrange("p (t s) -> p t s", t=NT),
                            axis=AX, op=MAX, negate=True)
                        p = pp.tile([128, LBLK], f32, tag="p")
                        if cfg["bias_mode"] == "act":
                            for ti in range(NT):
                                nc.scalar.activation(
                                    p[:, ts(ti, 128)], sc[:, ts(ti, 128)], Exp,
                                    bias=nm[:, ti:ti + 1], scale=1.0)
                        else:
                            nmt_ps = auxps.tile([NT, 128], f32, tag="nmt")
                            nc.tensor.transpose(nmt_ps[:], nm[:], ident[:])
                            nmt = nmp.tile([NT, 128], f32r, tag="nmtr")
                            if cfg["nm_copy_engine"] == "dve":
                                nc.vector.tensor_copy(nmt[:], nmt_ps[:])
                            else:
                                nc.scalar.copy(nmt[:], nmt_ps[:])
                            nc.tensor.matmul(sc[:], nmt[:], ind_r[:],
                                             start=False, stop=True,
                                             skip_group_check=True)
                            nc.scalar.activation(p[:], sc[:], Exp,
                                                 bias=0.0, scale=1.0)
                        if cfg["rowsum_mode"] == "dve":
                            nc.vector.tensor_reduce(
                                rs_stage[:, blk * NT:(blk + 1) * NT],
                                p[:].rearrange("p (t s) -> p t s", t=NT),
                                axis=AX, op=ADD)
                        ptps_t = ptps.tile([128, LBLK], f32, tag="ptps")
                        for ti in range(NT):
                            nc.tensor.transpose(
                                ptps_t[:, ts(ti, 128)], p[:, ts(ti, 128)],
                                ident[:])
                        pt = ptp.tile([128, LBLK], f32r, tag="pt")
                        pce = cfg["pt_copy_engine"]
                        if pce == "alt":
                            pce = "dve" if blk % 2 == 0 else "act"
                        elif pce.startswith("mix"):
                            n, m = pce[3:].split("of")
                            pce = "dve" if blk % int(m) < int(n) else "act"
                        if pce == "dve":
                            nc.vector.tensor_copy(pt[:], ptps_t[:])
                        else:
                            nc.scalar.copy(pt[:], ptps_t[:])
                        if cfg["rowsum_mode"] == "pool":
                            nc.gpsimd.partition_all_reduce(
                                rs_stage[:, blk * LBLK:(blk + 1) * LBLK],
                                pt[:], 128, bass_isa.ReduceOp.add)
                        elif cfg["rowsum_mode"] == "pe":
                            if blk % 4 == 0:
                                rs_ps = rspsp.tile([128, LBLK], f32, tag="rsps")
                            j = blk % 4
                            nc.tensor.matmul(
                                rs_ps[32 * j:32 * j + 1, :], ones_r[:], pt[:],
                                start=True, stop=True,
                                tile_position=(0, 32 * j))
                            if j == 3:
                                nc.vector.tensor_copy(
                                    rs_stage[(blk - 3) // 4 * 4:(blk - 3) // 4 * 4 + 4, :].rearrange("a b -> a b"),
                                    rs_ps[:].rearrange("(a c) b -> a c b", c=32)[:, 0:1, :].rearrange("a c b -> (a c) b"))
                        if cfg["rowsum_mode"] == "pe_pt":
                            if blk == 0:
                                rs16_new = rspsp.tile([NBLK, LBLK], f32,
                                                      tag="rsps")
                                rs16_holder[0] = rs16_new
                            rs16 = rs16_holder[0]
                            nc.tensor.matmul(
                                rs16[:],
                                sel16_r[:, blk * NBLK:(blk + 1) * NBLK],
                                pt[:], start=(blk == 0),
                                stop=(blk == NBLK - 1))
                            if blk == NBLK - 1:
                                rsx = rsp.tile([NBLK, LBLK], f32, tag="rsx")
                                nc.vector.tensor_copy(rsx[:], rs16[:])
                                nc.sync.dma_start(rs_d[b], rsx[:])
                        op_t = ops.tile([128, LBLK], f32, tag="op")
                        nc.tensor.matmul(
                            op_t[:], v_r[:, ts(b, 128)], pt[:],
                            start=True, stop=True)
                        oc = ocp.tile([128, LBLK], out_dt, tag="oc")
                        oce = cfg["out_copy_engine"]
                        if oce == "alt":
                            oce = "act" if blk % 2 == 0 else "dve"
                        elif oce.startswith("mix"):
                            n, m = oce[3:].split("of")
                            oce = "dve" if blk % int(m) < int(n) else "act"
                        if oce == "act":
                            nc.scalar.copy(oc[:], op_t[:])
                        else:
                            nc.vector.tensor_copy(oc[:], op_t[:])
                        st = {"pool": nc.gpsimd, "act": nc.scalar,
                              "dve": nc.vector, "sp": nc.sync}[cfg["store_engine"]]
                        st.dma_start(outT_d[b, :, l0:l0 + LBLK], oc[:])
                    if cfg["rowsum_mode"] == "dve":
                        nc.gpsimd.dma_start(rs_d[b], rs_stage[:])
                    elif cfg["rowsum_mode"] == "pool":
                        nc.sync.dma_start(rs_d[b], rs_stage[0:1, :].rearrange("a b -> (a b)"))

            if repeat == 1:
                body()
            else:
                with tc.For_i(0, repeat, 1) as _i:
                    body(_i)

    nc.compile()
    return nc


def _make_runner(repeat=1, cfg=None):
    """Compile (once) and return fn(in_maps) -> list[dict] per core."""
    key = (repeat, tuple(sorted((cfg or {}).items())))
    if key in _RUNNER_CACHE:
        return _RUNNER_CACHE[key]

    import jax
    import concourse.mybir as mybir
    from concourse import bass2jax
    from concourse.bass2jax import _bass_exec_p, partition_id_tensor
    from jax.sharding import Mesh, NamedSharding, PartitionSpec
    from jax.experimental.shard_map import shard_map

    nc = _build_nc(repeat, cfg)
    bass2jax.install_neuronx_cc_hook()

    in_names, out_names, out_avals, zero_shapes = [], [], [], []
    for alloc in nc.m.functions[0].allocations:
        if not isinstance(alloc, mybir.MemoryLocationSet):
            continue
        name = alloc.memorylocations[0].name
        if alloc.kind == "ExternalInput":
            if nc.partition_id_tensor is None or name != nc.partition_id_tensor.name:
                in_names.append(name)
        elif alloc.kind == "ExternalOutput":
            out_names.append(name)
            shape = tuple(alloc.tensor_shape)
            dtype = mybir.dt.np(alloc.dtype)
            out_avals.append(jax.core.ShapedArray(shape, dtype))
            zero_shapes.append((shape, dtype))
    n_params = len(in_names)
    pid_name = nc.partition_id_tensor.name if nc.partition_id_tensor else None
    names_for_bind = in_names + out_names + ([pid_name] if pid_name else [])

    def _body(*args):
        operands = list(args)
        if pid_name:
            operands.append(partition_id_tensor())
        outs = _bass_exec_p.bind(
            *operands,
            out_avals=tuple(out_avals),
            in_names=tuple(names_for_bind),
            out_names=tuple(out_names),
            lowering_input_output_aliases=(),
            sim_require_finite=True,
            sim_require_nnan=True,
            nc=nc,
        )
        return tuple(outs)

    devices = jax.devices()[:N_CORES]
    mesh = Mesh(np.asarray(devices), ("core",))
    nspec = n_params + len(out_names)
    fn = jax.jit(
        shard_map(_body, mesh=mesh,
                  in_specs=(PartitionSpec("core"),) * nspec,
                  out_specs=(PartitionSpec("core"),) * len(out_names),
                  check_rep=False),
        keep_unused=True)
    sharding = NamedSharding(mesh, PartitionSpec("core"))

    def run(in_maps):
        import jax as _jax
        concat_in = [
            np.concatenate([np.asarray(m[name]) for m in in_maps], axis=0)
            for name in in_names
        ]
        zeros = [np.zeros((N_CORES * s[0],) + tuple(s[1:]), d)
                 for (s, d) in zero_shapes]
        dev_in = [_jax.device_put(a, sharding) for a in concat_in + zeros]
        out_arrs = fn(*dev_in)
        _jax.block_until_ready(out_arrs)
        return [
            {name: np.asarray(out_arrs[i]).reshape(
                (N_CORES,) + tuple(out_avals[i].shape))[c]
             for i, name in enumerate(out_names)}
            for c in range(N_CORES)
        ], (fn, dev_in)

    _RUNNER_CACHE[key] = run
    return run


def _prep_inputs_t2(queries, keys, values, cfg=None):
    cfg = {**CFG, **(cfg or {})}
    import ml_dtypes
    bf = ml_dtypes.bfloat16
    q = np.asarray(queries, np.float32)
    k = np.asarray(keys, np.float32)
    v = np.asarray(values, np.float32)
    mmdt = np.float16 if cfg["t2_mm_dtype"] == "f16" else bf
    qT16 = np.ascontiguousarray(q.transpose(0, 2, 1).astype(mmdt))
    k16 = np.ascontiguousarray(k.astype(mmdt))                 # (B, e, j)
    v16 = np.ascontiguousarray(v.astype(bf))                   # (B, j, d)
    qsum = q.sum(axis=2)                                       # (B, L) f32
    krow = k.sum(axis=2)                                       # (B, S) f32
    f16 = mmdt
    qh = qsum.astype(f16)
    ql = (qsum - qh.astype(np.float32)).astype(f16)
    kh = krow.astype(f16)
    kl = (krow - kh.astype(np.float32)).astype(f16)
    c = np.maximum(qsum * krow.max(axis=1)[:, None],
                   qsum * krow.min(axis=1)[:, None])           # (B, L)
    cb = c.astype(f16)
    ones = np.ones((B, S), np.float32).astype(f16)
    alhs = np.ascontiguousarray(
        np.stack([kh, kl, kh, -ones], axis=1))                 # (B, 4, S)
    arhs = np.ascontiguousarray(
        np.stack([qh, qh, ql, cb], axis=1))                    # (B, 4, L)
    sel = np.zeros((S, NBLK * NBLK), np.float32)
    for j in range(NBLK):
        sel[:, j * NBLK + j] = 1.0
    selb = sel.astype(bf)
    in_maps = []
    for ci in range(N_CORES):
        sl = slice(ci * B_LOC, (ci + 1) * B_LOC)
        in_maps.append({"qT16": qT16[sl], "k16": k16[sl], "v16": v16[sl],
                        "alhs": alhs[sl], "arhs": arhs[sl], "selb": selb})
    return in_maps


def _assemble_t2(results):
    out = np.empty((B, L, S), dtype=np.float32)
    for ci in range(N_CORES):
        outT = results[ci]["outT"]         # (B_LOC, S, L) bf16, = out^T
        rs = results[ci]["rs"]             # (B_LOC, NBLK, LBLK) f32
        for b in range(B_LOC):
            rsum = rs[b].reshape(L)
            out[ci * B_LOC + b] = outT[b].astype(np.float32).T / rsum[:, None]
    return out.reshape(B, 1, L, S)


def _prep_inputs(queries, keys, values, cfg=None):
    cfg = {**CFG, **(cfg or {})}
    if cfg["dataflow"] == "t2":
        return _prep_inputs_t2(queries, keys, values, cfg)
    qT = np.ascontiguousarray(queries.transpose(0, 2, 1))      # (B, E, L)
    kp = keys + keys.sum(axis=2)[:, None, :]                   # k' = k + 1*ksum
    kp = np.ascontiguousarray(kp.astype(np.float32))
    v = np.ascontiguousarray(values.astype(np.float32))
    qT2 = kph = kpl = None
    if cfg["mm1_dtype"] == "bf16x2":
        import ml_dtypes
        bf = ml_dtypes.bfloat16
        qTh = qT.astype(bf)
        qTl = (qT - qTh.astype(np.float32)).astype(bf)
        qT2 = np.ascontiguousarray(np.stack([qTh, qTl], axis=2))
        kph = kp.astype(bf)
        kpl = np.ascontiguousarray((kp - kph.astype(np.float32)).astype(bf))
        kph = np.ascontiguousarray(kph)
    ind = np.zeros((NT, LBLK), np.float32)
    for ti in range(NT):
        ind[ti, ti * 128:(ti + 1) * 128] = 1.0
    sel16 = np.zeros((S, NBLK * NBLK), np.float32)
    for j in range(NBLK):
        sel16[:, j * NBLK + j] = 1.0
    in_maps = []
    for c in range(N_CORES):
        sl = slice(c * B_LOC, (c + 1) * B_LOC)
        m = {"qT": qT[sl], "kp": kp[sl], "v": v[sl], "ind": ind,
             "sel16": sel16}
        if qT2 is not None:
            m.update({"qT2": qT2[sl], "kph": kph[sl], "kpl": kpl[sl]})
        in_maps.append(m)
    return in_maps


def _assemble(results, cfg=None):
    cfg = {**CFG, **(cfg or {})}
    if cfg["dataflow"] == "t2":
        return _assemble_t2(results)
    out = np.empty((B, L, S), dtype=np.float32)
    for c in range(N_CORES):
        outT = results[c]["outT"]          # (B_LOC, S, L)  = out^T per batch
        rs = results[c]["rs"]              # (B_LOC, 128, L//128) rowsums
        for b in range(B_LOC):
            if cfg["rowsum_mode"] == "dve" and cfg["dataflow"] == "lsoft":
                rsum = rs[b].T.reshape(L)  # rowsum[l]
            else:
                rsum = rs[b].reshape(L)
            out[c * B_LOC + b] = outT[b].T / rsum[:, None]
    return out.reshape(B, 1, L, S)


def kernel(queries, keys, values):
    run = _make_runner(repeat=1)
    in_maps = _prep_inputs(queries, keys, values)
    results, _ = run(in_maps)
    return _assemble(results)



# revision 22
# speedup vs baseline: 1.4254x; 1.1692x over previous
"""Trainium2 Bass kernel for an AxialAttentionLayer-style module.

Math: for each batch b,
    scores = q @ k'          where k'[e,j] = keys[e,j] + sum_d keys[j,d]
    A      = softmax(scores, axis=-1)
    out    = A @ values
(the reference's rank-1 additive score s1 folds into the matmul because
 s1[l,j] = (sum_e q[l,e]) * ksum[j] = sum_e q[l,e]*ksum[j]).

Sharding: data-parallel over batch B=32 across 8 cores (4 batches/core).
Device layout per core, per 512-row block of L:
    MM1   (PE, fp32):   scores(l,s) = qT_chunk.T @ k'   (4x 128-tiles)
    max   (DVE):        fused 3D reduce_max(negate) -> -rowmax (128,4)
    exp   (ACT):        P = exp(scores - rowmax), per-tile bias
    rowsum(DVE):        fused 3D reduce_sum over P -> staged per batch
    T     (PE):         P chunks transposed via PE transpose -> PSUM
    copy  (ACT):        PT PSUM -> SBUF (rounded to fp32r)
    MM2   (PE, fp32r):  outT(d,l) = v.T-free matmul with v stationary
    copy  (DVE):        outT PSUM -> SBUF
Host: pre-transposes q -> qT, builds k', divides by rowsum, transposes back.
"""

import numpy as np

B, L, S = 32, 8192, 128
N_CORES = 8
B_LOC = B // N_CORES  # 4
LBLK = 512            # l-rows per block
NT = LBLK // 128      # 128-tiles per block
NBLK = L // LBLK      # blocks per batch

_RUNNER_CACHE = {}

# tunables (overridable before building)
CFG = dict(
    qt_bufs=8, p_bufs=4, pt_bufs=4, nm_bufs=4, oc_bufs=6, rs_bufs=2,
    sc_bufs=4, ptps_bufs=2, o_bufs=2,
    out_copy_engine="dve",   # "act" | "dve" | "alt"
    pt_copy_engine="act",    # "act" | "dve" | "alt"
    store_engine="sp",       # "pool" | "act" | "dve" | "sp"
    rowsum_mode="dve",       # "dve" | "pool" | "pe"
    mm1_dtype="f32",         # "f32" | "f32r" | "bf16x2"
    out_dtype="f32",         # "f32" | "bf16"
    bias_mode="act",         # "act" (per-tile exp bias) | "pe" (K=4 accum matmul)
    nm_copy_engine="act",    # "dve" | "act"
    scs_copy_engine="dve",   # "dve" | "act"
    dataflow="t2",           # "lsoft" | "t" | "t2" (fp16 MM1 + host shift)
    rsps_bufs=1,
    max_out_dtype="f32r",    # partition_all_reduce out dtype in "t" flow
    # --- t2 knobs ---
    t2_rowsum="pe",          # "pe" (grouped sel16) | "pes"/"pe1"/"pool" (broken here)
    t2_oc_engine="dve",      # "dve" | "act"
    t2_rs_copy="dve",        # "dve" | "act" | "pool"
    t2_sc_bufs=2, t2_o_bufs=2, t2_rsps_bufs=2,
    t2_qs_bufs=2, t2_ot_bufs=2, t2_q_dmas=2, t2_o_dmas=1,
    t2_mm_dtype="f16",       # "f16" | "bf16" (bf16: timing diagnostics only)
    t2_skip_rowsum=False,    # True: omit rowsum matmul+DMA (diagnostics)
    t2_skip_additive=False,  # True: omit additive matmul (diagnostics)
    t2_swp=True,             # software-pipeline PE stream across blocks
    t2_additive_mode="mul",  # "mul" (exp-split) | "pe" | "pre" | "sep" (diag)
)


def _build_nc_t2(repeat=1, cfg=None):
    """fp16 MM1 + K=4 bf16 additive (rank-1 score + host softmax shift),
    no device max, bf16 P/V/out. Layout: scores in (j part, l free)."""
    cfg = {**CFG, **(cfg or {})}
    import concourse.bacc as bacc
    import concourse.mybir as mybir
    import concourse.tile as tile
    from concourse.bass import ts
    from concourse import bass_isa

    f32 = mybir.dt.float32
    f16 = mybir.dt.float16 if cfg["t2_mm_dtype"] == "f16" else mybir.dt.bfloat16
    bf16 = mybir.dt.bfloat16
    Exp = mybir.ActivationFunctionType.Exp

    nc = bacc.Bacc("TRN2", target_bir_lowering=False, debug=False)
    qT_d = nc.dram_tensor("qT16", (B_LOC, S, L), f16, kind="ExternalInput")
    k_d = nc.dram_tensor("k16", (B_LOC, S, S), f16, kind="ExternalInput")
    v_d = nc.dram_tensor("v16", (B_LOC, S, S), bf16, kind="ExternalInput")
    alhs_d = nc.dram_tensor("alhs", (B_LOC, 4, S), f16, kind="ExternalInput")
    arhs_d = nc.dram_tensor("arhs", (B_LOC, 4, L), f16, kind="ExternalInput")
    sel_d = nc.dram_tensor("selb", (S, NBLK * NBLK), bf16, kind="ExternalInput")
    ones_d = None
    if cfg["t2_rowsum"] == "pe1":
        ones_d = nc.dram_tensor("onesb", (S, 1), bf16, kind="ExternalInput")
    outT_d = nc.dram_tensor("outT", (B_LOC, S, L), bf16, kind="ExternalOutput")
    rs_d = nc.dram_tensor("rs", (B_LOC, NBLK, LBLK), f32, kind="ExternalOutput")

    NQD = cfg["t2_q_dmas"]     # DMAs per batch for q load
    NOD = cfg["t2_o_dmas"]     # DMAs per batch for out store
    with tile.TileContext(nc) as tc:
        with (
            tc.tile_pool(name="const", bufs=1) as constp,
            tc.tile_pool(name="qs", bufs=cfg["t2_qs_bufs"]) as qsp,
            tc.tile_pool(name="ot", bufs=cfg["t2_ot_bufs"]) as otp,
            tc.tile_pool(name="pt", bufs=cfg["pt_bufs"]) as ptp,
            tc.tile_pool(name="rss", bufs=cfg["rs_bufs"]) as rsp,
            tc.tile_pool(name="scps", bufs=cfg["t2_sc_bufs"], space="PSUM") as scps,
            tc.tile_pool(name="ops", bufs=cfg["t2_o_bufs"], space="PSUM") as ops,
            tc.tile_pool(name="rsps", bufs=cfg["t2_rsps_bufs"], space="PSUM") as rspsp,
            tc.tile_pool(name="dump", bufs=2) as dumpp,
            tc.tile_pool(name="adps", bufs=2, space="PSUM") as adpsp,
            tc.tile_pool(name="t12", bufs=6) as t12p,
        ):
            k_sb = constp.tile([128, B_LOC * 128], f16, tag="k16")
            v_sb = constp.tile([128, B_LOC * 128], bf16, tag="v16")
            alhs_sb = constp.tile([4, B_LOC * 128], f16, tag="alhs")
            arc_sb = constp.tile([4, B_LOC * L], f16, tag="arc")
            sel_sb = constp.tile([128, NBLK * NBLK], bf16, tag="selb")
            nc.sync.dma_start(sel_sb[:], sel_d[:])
            ones_sb = None
            if cfg["t2_rowsum"] == "pe1":
                ones_sb = constp.tile([128, 1], bf16, tag="ones_b")
                nc.sync.dma_start(ones_sb[:], ones_d[:])
            for b in range(B_LOC):
                nc.sync.dma_start(k_sb[:, ts(b, 128)], k_d[b])
                nc.sync.dma_start(v_sb[:, ts(b, 128)], v_d[b])
                nc.sync.dma_start(alhs_sb[:, ts(b, 128)], alhs_d[b])
                nc.sync.dma_start(arc_sb[:, b * L:(b + 1) * L], arhs_d[b])

            state = {}

            def stage_mm1(b, blk):
                l0 = blk * LBLK
                mode = cfg["t2_additive_mode"]
                if cfg["t2_skip_additive"]:
                    mode = "none"
                arc_sl = arc_sb[:, b * L + l0:b * L + l0 + LBLK]
                sc = scps.tile([128, LBLK], f32, tag="sc")
                scr = None
                if mode == "pre":
                    nc.tensor.matmul(sc[:], alhs_sb[:, ts(b, 128)], arc_sl,
                                     start=True, stop=False)
                nc.tensor.matmul(sc[:], k_sb[:, ts(b, 128)],
                                 qs_cur[0][:, l0:l0 + LBLK],
                                 start=(mode != "pre"),
                                 stop=(mode in ("none", "sep", "mul", "pre")))
                if mode == "pe":
                    nc.tensor.matmul(sc[:], alhs_sb[:, ts(b, 128)], arc_sl,
                                     start=False, stop=True)
                elif mode == "sep":
                    scr = rspsp.tile([128, LBLK], f32, tag="scr")
                    nc.tensor.matmul(scr[:], alhs_sb[:, ts(b, 128)], arc_sl,
                                     start=True, stop=True)
                    dmp = dumpp.tile([1, 4], f32, tag="dmp")
                    nc.vector.tensor_copy(dmp[:], scr[0:1, 0:4])
                    scr = None
                elif mode == "mul":
                    scr = adpsp.tile([128, LBLK], f32, tag="scr")
                    nc.tensor.matmul(scr[:], alhs_sb[:, ts(b, 128)], arc_sl,
                                     start=True, stop=True)
                return sc, scr

            def stage_rest(b, blk, scpair):
                sc, scr = scpair
                l0 = blk * LBLK
                pt = ptp.tile([128, LBLK], bf16, tag="pt")
                if scr is not None:
                    import concourse.mybir as _mb
                    t1 = t12p.tile([128, LBLK], bf16, tag="t1")
                    nc.scalar.activation(t1[:], sc[:], Exp, bias=0.0, scale=1.0)
                    t2 = t12p.tile([128, LBLK], bf16, tag="t2")
                    nc.scalar.activation(t2[:], scr[:], Exp, bias=0.0, scale=1.0)
                    nc.vector.scalar_tensor_tensor(
                        pt[:], t1[:], 1.0, t2[:],
                        op0=_mb.AluOpType.mult, op1=_mb.AluOpType.mult)
                else:
                    nc.scalar.activation(pt[:], sc[:], Exp, bias=0.0, scale=1.0)
                rsmode = ("skip" if cfg["t2_skip_rowsum"] else cfg["t2_rowsum"])
                if rsmode == "pes":
                    if blk == 0:
                        rsx_new = rsp.tile([NBLK, LBLK], f32, tag="rsxs")
                        state["rsx"] = rsx_new
                    rsf = rspsp.tile([NBLK, LBLK], f32, tag="rsf")
                    nc.tensor.matmul(
                        rsf[:], sel_sb[:, blk * NBLK:(blk + 1) * NBLK],
                        pt[:], start=True, stop=True)
                    nc.vector.tensor_copy(
                        state["rsx"][blk:blk + 1, :], rsf[blk:blk + 1, :])
                    if blk == NBLK - 1:
                        nc.sync.dma_start(rs_d[b], state["rsx"][:])
                elif rsmode == "pe1":
                    if blk == 0:
                        rsx_new = rsp.tile([NBLK, LBLK], f32, tag="rsxs")
                        state["rsx"] = rsx_new
                    rs1 = rspsp.tile([1, LBLK], f32, tag="rs1")
                    nc.tensor.matmul(rs1[:], ones_sb[:], pt[:],
                                     start=True, stop=True)
                    nc.vector.tensor_copy(
                        state["rsx"][blk:blk + 1, :], rs1[:])
                    if blk == NBLK - 1:
                        nc.sync.dma_start(rs_d[b], state["rsx"][:])
                elif rsmode == "pe":
                    if blk == 0:
                        rs16_new = rspsp.tile([NBLK, LBLK], f32, tag="rsps")
                        state["rs16"] = rs16_new
                    nc.tensor.matmul(
                        state["rs16"][:],
                        sel_sb[:, blk * NBLK:(blk + 1) * NBLK],
                        pt[:], start=(blk == 0), stop=(blk == NBLK - 1))
                elif rsmode == "pool":
                    prt = rsp.tile([128, LBLK], f32, tag="prt")
                    nc.gpsimd.partition_all_reduce(
                        prt[:], pt[:], 128, bass_isa.ReduceOp.add)
                    nc.gpsimd.dma_start(
                        rs_d[b, blk], prt[0:1, :].rearrange("a b -> (a b)"))
                op_t = ops.tile([128, LBLK], f32, tag="op")
                nc.tensor.matmul(op_t[:], v_sb[:, ts(b, 128)], pt[:],
                                 start=True, stop=True)
                if cfg["t2_oc_engine"] == "act":
                    nc.scalar.copy(ot_cur[0][:, l0:l0 + LBLK], op_t[:])
                else:
                    nc.vector.tensor_copy(ot_cur[0][:, l0:l0 + LBLK], op_t[:])
                if blk == NBLK - 1:
                    if rsmode == "pe":
                        rsx = rsp.tile([NBLK, LBLK], f32, tag="rsx")
                        if cfg["t2_rs_copy"] == "act":
                            nc.scalar.copy(rsx[:], state["rs16"][:])
                        else:
                            nc.vector.tensor_copy(rsx[:], state["rs16"][:])
                        nc.sync.dma_start(rs_d[b], rsx[:])
                    HO = L // NOD
                    for h in range(NOD):
                        nc.sync.dma_start(
                            outT_d[b, :, h * HO:(h + 1) * HO],
                            ot_cur[0][:, h * HO:(h + 1) * HO])

            qs_cur = [None]
            ot_cur = [None]

            def load_batch(b):
                qs = qsp.tile([128, L], f16, tag="qs")
                H = L // NQD
                for h in range(NQD):
                    nc.sync.dma_start(qs[:, h * H:(h + 1) * H],
                                      qT_d[b, :, h * H:(h + 1) * H])
                return qs

            def body(_iv=None):
                if not cfg["t2_swp"]:
                    for b in range(B_LOC):
                        qs_cur[0] = load_batch(b)
                        ot_new = otp.tile([128, L], bf16, tag="ot")
                        ot_cur[0] = ot_new
                        for blk in range(NBLK):
                            scpair = stage_mm1(b, blk)
                            stage_rest(b, blk, scpair)
                    return
                # software-pipelined: PE runs mm1(i+1) before rs/mm2(i)
                items = [(b, blk) for b in range(B_LOC)
                         for blk in range(NBLK)]
                pend = None
                for i, (b, blk) in enumerate(items):
                    if blk == 0:
                        if b == 0:
                            qs_cur[0] = load_batch(b)
                        state[("qs_next", b)] = None
                        ot_new = otp.tile([128, L], bf16, tag="ot")
                        state[("ot", b)] = ot_new
                    if blk == 1 and b + 1 < B_LOC:
                        state[("qs_next", b)] = load_batch(b + 1)
                    scpair = stage_mm1(b, blk)
                    if pend is not None:
                        pb, pblk, psc, pqs, pot = pend
                        qs_sav, ot_sav = qs_cur[0], ot_cur[0]
                        qs_cur[0], ot_cur[0] = pqs, pot
                        stage_rest(pb, pblk, psc)
                        qs_cur[0], ot_cur[0] = qs_sav, ot_sav
                    pend = (b, blk, scpair, qs_cur[0], state[("ot", b)])
                    if blk == NBLK - 1 and b + 1 < B_LOC:
                        qs_cur[0] = state[("qs_next", b)]
                pb, pblk, psc, pqs, pot = pend
                qs_cur[0], ot_cur[0] = pqs, pot
                stage_rest(pb, pblk, psc)

            if repeat == 1:
                body()
            else:
                with tc.For_i(0, repeat, 1) as _i:
                    body(_i)

    nc.compile()
    return nc


def _build_nc(repeat=1, cfg=None):
    cfg = {**CFG, **(cfg or {})}
    if cfg["dataflow"] == "t2":
        return _build_nc_t2(repeat, cfg)
    import concourse.bacc as bacc
    import concourse.mybir as mybir
    import concourse.tile as tile
    from concourse.bass import ts
    from concourse.masks import make_identity

    f32 = mybir.dt.float32
    f32r = mybir.dt.float32r

    nc = bacc.Bacc("TRN2", target_bir_lowering=False, debug=False)
    bf16 = mybir.dt.bfloat16
    if cfg["mm1_dtype"] == "bf16x2":
        qT_d = nc.dram_tensor("qT2", (B_LOC, S, 2, L), bf16, kind="ExternalInput")
        kph_d = nc.dram_tensor("kph", (B_LOC, S, S), bf16, kind="ExternalInput")
        kpl_d = nc.dram_tensor("kpl", (B_LOC, S, S), bf16, kind="ExternalInput")
    else:
        mm1_dt_glob = f32 if cfg["mm1_dtype"] == "f32" else f32r
        qT_d = nc.dram_tensor("qT", (B_LOC, S, L), mm1_dt_glob, kind="ExternalInput")
    kp_d = nc.dram_tensor("kp", (B_LOC, S, S), f32, kind="ExternalInput")
    v_d = nc.dram_tensor("v", (B_LOC, S, S), f32, kind="ExternalInput")
    ind_d = None
    if cfg["bias_mode"] == "pe":
        ind_d = nc.dram_tensor("ind", (NT, LBLK), f32r, kind="ExternalInput")
    sel16_d = None
    if cfg["dataflow"] == "t" or cfg["rowsum_mode"] == "pe_pt":
        sel16_d = nc.dram_tensor("sel16", (S, NBLK * NBLK), f32r,
                                 kind="ExternalInput")
    out_dt = f32 if cfg["out_dtype"] == "f32" else mybir.dt.bfloat16
    outT_d = nc.dram_tensor("outT", (B_LOC, S, L), out_dt, kind="ExternalOutput")
    if cfg["rowsum_mode"] == "dve" and cfg["dataflow"] == "lsoft":
        rs_d = nc.dram_tensor("rs", (B_LOC, S, L // S), f32, kind="ExternalOutput")
    elif cfg["dataflow"] == "t" or cfg["rowsum_mode"] == "pe_pt":
        rs_d = nc.dram_tensor("rs", (B_LOC, NBLK, LBLK), f32, kind="ExternalOutput")
    else:
        rs_d = nc.dram_tensor("rs", (B_LOC, L), f32, kind="ExternalOutput")

    from concourse import bass_isa
    Exp = mybir.ActivationFunctionType.Exp
    AX = mybir.AxisListType.X
    MAX = mybir.AluOpType.max
    ADD = mybir.AluOpType.add

    with tile.TileContext(nc) as tc:
        with (
            tc.tile_pool(name="const", bufs=1) as constp,
            tc.tile_pool(name="qt", bufs=cfg["qt_bufs"]) as qtp,
            tc.tile_pool(name="p", bufs=cfg["p_bufs"]) as pp,
            tc.tile_pool(name="pt", bufs=cfg["pt_bufs"]) as ptp,
            tc.tile_pool(name="nm", bufs=cfg["nm_bufs"]) as nmp,
            tc.tile_pool(name="rss", bufs=cfg["rs_bufs"]) as rsp,
            tc.tile_pool(name="oc", bufs=cfg["oc_bufs"]) as ocp,
            tc.tile_pool(name="scps", bufs=cfg["sc_bufs"], space="PSUM") as scps,
            tc.tile_pool(name="ptps", bufs=cfg["ptps_bufs"], space="PSUM") as ptps,
            tc.tile_pool(name="ops", bufs=cfg["o_bufs"], space="PSUM") as ops,
            tc.tile_pool(name="rsps", bufs=cfg["rsps_bufs"], space="PSUM") as rspsp,
            tc.tile_pool(name="auxps", bufs=1, space="PSUM") as auxps,
        ):
            ident = constp.tile([128, 128], f32, tag="ident")
            make_identity(nc, ident[:])
            kp_sb = constp.tile([128, B_LOC * 128], f32, tag="kp")
            v_sb = constp.tile([128, B_LOC * 128], f32, tag="v")
            v_r = constp.tile([128, B_LOC * 128], f32r, tag="vr")
            ind_r = None
            if cfg["bias_mode"] == "pe":
                ind_r = constp.tile([NT, LBLK], f32r, tag="ind")
                nc.sync.dma_start(ind_r[:], ind_d[:])
            ones_r = None
            if cfg["rowsum_mode"] == "pe" or cfg["dataflow"] == "t":
                ones_f = constp.tile([128, 1], f32, tag="ones_f")
                ones_r = constp.tile([128, 1], f32r, tag="ones")
                nc.gpsimd.memset(ones_f[:], 1.0)
                nc.vector.tensor_copy(ones_r[:], ones_f[:])
            neg_inv_r = None
            sel16_r = None
            if cfg["rowsum_mode"] == "pe_pt" and cfg["dataflow"] != "t":
                sel16_r = constp.tile([128, NBLK * NBLK], f32r, tag="sel16")
                nc.sync.dma_start(sel16_r[:], sel16_d[:])
            if cfg["dataflow"] == "t":
                neg_inv_f = constp.tile([128, 128], f32, tag="ninv_f")
                neg_inv_r = constp.tile([128, 128], f32r, tag="ninv")
                nc.gpsimd.memset(neg_inv_f[:], -1.0 / 128.0)
                nc.vector.tensor_copy(neg_inv_r[:], neg_inv_f[:])
                sel16_r = constp.tile([128, NBLK * NBLK], f32r, tag="sel16")
                nc.sync.dma_start(sel16_r[:], sel16_d[:])
            for b in range(B_LOC):
                nc.sync.dma_start(kp_sb[:, ts(b, 128)], kp_d[b])
                nc.sync.dma_start(v_sb[:, ts(b, 128)], v_d[b])
            nc.vector.tensor_copy(v_r[:], v_sb[:])
            kp_r = None
            if cfg["mm1_dtype"] == "f32r":
                kp_r = constp.tile([128, B_LOC * 128], f32r, tag="kpr")
                nc.vector.tensor_copy(kp_r[:], kp_sb[:])
            kph_sb = kpl_sb = None
            if cfg["mm1_dtype"] == "bf16x2":
                bf16_ = mybir.dt.bfloat16
                kph_sb = constp.tile([128, B_LOC * 128], bf16_, tag="kph")
                kpl_sb = constp.tile([128, B_LOC * 128], bf16_, tag="kpl")
                for b in range(B_LOC):
                    nc.sync.dma_start(kph_sb[:, ts(b, 128)], kph_d[b])
                    nc.sync.dma_start(kpl_sb[:, ts(b, 128)], kpl_d[b])

            def t_block(b, blk, rs_stage, rs_ps_holder):
                l0 = blk * LBLK
                sc = scps.tile([128, LBLK], f32, tag="sc")
                if cfg["mm1_dtype"] == "bf16x2":
                    bf16_ = mybir.dt.bfloat16
                    qt2 = qtp.tile([128, 2 * LBLK], bf16_, tag="qt")
                    nc.sync.dma_start(
                        qt2[:].rearrange("p (h l) -> p h l", h=2),
                        qT_d[b, :, :, l0:l0 + LBLK])
                    qh = qt2[:, 0:LBLK]
                    ql = qt2[:, LBLK:2 * LBLK]
                    nc.tensor.matmul(sc[:], kph_sb[:, ts(b, 128)], qh,
                                     start=True, stop=False)
                    nc.tensor.matmul(sc[:], kpl_sb[:, ts(b, 128)], qh,
                                     start=False, stop=False)
                    nc.tensor.matmul(sc[:], kph_sb[:, ts(b, 128)], ql,
                                     start=False, stop=False)
                else:
                    mm1_dt = f32 if cfg["mm1_dtype"] == "f32" else f32r
                    kp_use = kp_sb if cfg["mm1_dtype"] == "f32" else kp_r
                    qt = qtp.tile([128, LBLK], mm1_dt, tag="qt")
                    nc.sync.dma_start(qt[:], qT_d[b, :, l0:l0 + LBLK])
                    nc.tensor.matmul(sc[:], kp_use[:, ts(b, 128)], qt[:],
                                     start=True, stop=False)
                scs = pp.tile([128, LBLK], f32, tag="scs")
                if cfg["scs_copy_engine"] == "dve":
                    nc.vector.tensor_copy(scs[:], sc[:])
                else:
                    nc.scalar.copy(scs[:], sc[:])
                mx_dt = f32r if cfg["max_out_dtype"] == "f32r" else f32
                mxr = ptp.tile([128, LBLK], mx_dt, tag="mxr")
                nc.gpsimd.partition_all_reduce(
                    mxr[:], scs[:], 128, bass_isa.ReduceOp.max)
                nc.tensor.matmul(sc[:], neg_inv_r[:], mxr[:],
                                 start=False, stop=True)
                pt = ptp.tile([128, LBLK], f32r, tag="pt")
                nc.scalar.activation(pt[:], sc[:], Exp, bias=0.0, scale=1.0)
                if blk == 0:
                    rs_ps_new = rspsp.tile([NBLK, LBLK], f32, tag="rsps")
                    rs_ps_holder[0] = rs_ps_new
                rs_ps = rs_ps_holder[0]
                nc.tensor.matmul(rs_ps[:], sel16_r[:, blk * NBLK:(blk + 1) * NBLK],
                                 pt[:], start=(blk == 0), stop=(blk == NBLK - 1))
                if blk == NBLK - 1:
                    rsx = rsp.tile([NBLK, LBLK], f32, tag="rsx")
                    nc.vector.tensor_copy(rsx[:], rs_ps[:])
                    nc.sync.dma_start(rs_d[b], rsx[:])
                op_t = ops.tile([128, LBLK], f32, tag="op")
                nc.tensor.matmul(op_t[:], v_r[:, ts(b, 128)], pt[:],
                                 start=True, stop=True)
                oc = ocp.tile([128, LBLK], out_dt, tag="oc")
                oce = cfg["out_copy_engine"]
                if oce in ("alt", "act") or oce.startswith("mix"):
                    nc.scalar.copy(oc[:], op_t[:])
                else:
                    nc.vector.tensor_copy(oc[:], op_t[:])
                st = {"pool": nc.gpsimd, "act": nc.scalar,
                      "dve": nc.vector, "sp": nc.sync}[cfg["store_engine"]]
                st.dma_start(outT_d[b, :, l0:l0 + LBLK], oc[:])

            def t_body(_iv=None):
                for b in range(B_LOC):
                    holder = [None]
                    for blk in range(NBLK):
                        t_block(b, blk, None, holder)

            def body(_iv=None):
                if cfg["dataflow"] == "t":
                    return t_body(_iv)
                for b in range(B_LOC):
                    mode = cfg["rowsum_mode"]
                    rs16_holder = [None]
                    rs_stage = None
                    if mode == "dve":
                        rs_stage = rsp.tile([128, L // S], f32, tag="rss")
                    elif mode == "pool":
                        rs_stage = rsp.tile([128, L], f32, tag="rss")
                    for blk in range(NBLK):
                        l0 = blk * LBLK
                        mm1_dt = f32 if cfg["mm1_dtype"] == "f32" else f32r
                        qt = qtp.tile([128, LBLK], mm1_dt, tag="qt")
                        nc.sync.dma_start(qt[:], qT_d[b, :, l0:l0 + LBLK])
                        sc = scps.tile([128, LBLK], f32, tag="sc")
                        for ti in range(NT):
                            nc.tensor.matmul(
                                sc[:, ts(ti, 128)], qt[:, ts(ti, 128)],
                                (kp_sb if cfg["mm1_dtype"] == "f32" else kp_r)[:, ts(b, 128)],
                                start=True,
                                stop=(cfg["bias_mode"] == "act"),
                                skip_group_check=(cfg["bias_mode"] == "pe"))
                        nm = nmp.tile([128, NT], f32, tag="nm")
                        nc.vector.tensor_reduce(
                            nm[:], sc[:].rearrange("p (t s) -> p t s", t=NT),
                            axis=AX, op=MAX, negate=True)
                        p = pp.tile([128, LBLK], f32, tag="p")
                        if cfg["bias_mode"] == "act":
                            for ti in range(NT):
                                nc.scalar.activation(
                                    p[:, ts(ti, 128)], sc[:, ts(ti, 128)], Exp,
                                    bias=nm[:, ti:ti + 1], scale=1.0)
                        else:
                            nmt_ps = auxps.tile([NT, 128], f32, tag="nmt")
                            nc.tensor.transpose(nmt_ps[:], nm[:], ident[:])
                            nmt = nmp.tile([NT, 128], f32r, tag="nmtr")
                            if cfg["nm_copy_engine"] == "dve":
                                nc.vector.tensor_copy(nmt[:], nmt_ps[:])
                            else:
                                nc.scalar.copy(nmt[:], nmt_ps[:])
                            nc.tensor.matmul(sc[:], nmt[:], ind_r[:],
                                             start=False, stop=True,
                                             skip_group_check=True)
                            nc.scalar.activation(p[:], sc[:], Exp,
                                                 bias=0.0, scale=1.0)
                        if cfg["rowsum_mode"] == "dve":
                            nc.vector.tensor_reduce(
                                rs_stage[:, blk * NT:(blk + 1) * NT],
                                p[:].rearrange("p (t s) -> p t s", t=NT),
                                axis=AX, op=ADD)
                        ptps_t = ptps.tile([128, LBLK], f32, tag="ptps")
                        for ti in range(NT):
                            nc.tensor.transpose(
                                ptps_t[:, ts(ti, 128)], p[:, ts(ti, 128)],
                                ident[:])
                        pt = ptp.tile([128, LBLK], f32r, tag="pt")
                        pce = cfg["pt_copy_engine"]
                        if pce == "alt":
                            pce = "dve" if blk % 2 == 0 else "act"
                        elif pce.startswith("mix"):
                            n, m = pce[3:].split("of")
                            pce = "dve" if blk % int(m) < int(n) else "act"
                        if pce == "dve":
                            nc.vector.tensor_copy(pt[:], ptps_t[:])
                        else:
                            nc.scalar.copy(pt[:], ptps_t[:])
                        if cfg["rowsum_mode"] == "pool":
                            nc.gpsimd.partition_all_reduce(
                                rs_stage[:, blk * LBLK:(blk + 1) * LBLK],
                                pt[:], 128, bass_isa.ReduceOp.add)
                        elif cfg["rowsum_mode"] == "pe":
                            if blk % 4 == 0:
                                rs_ps = rspsp.tile([128, LBLK], f32, tag="rsps")
                            j = blk % 4
                            nc.tensor.matmul(
                                rs_ps[32 * j:32 * j + 1, :], ones_r[:], pt[:],
                                start=True, stop=True,
                                tile_position=(0, 32 * j))
                            if j == 3:
                                nc.vector.tensor_copy(
                                    rs_stage[(blk - 3) // 4 * 4:(blk - 3) // 4 * 4 + 4, :].rearrange("a b -> a b"),
                                    rs_ps[:].rearrange("(a c) b -> a c b", c=32)[:, 0:1, :].rearrange("a c b -> (a c) b"))
                        if cfg["rowsum_mode"] == "pe_pt":
                            if blk == 0:
                                rs16_new = rspsp.tile([NBLK, LBLK], f32,
                                                      tag="rsps")
                                rs16_holder[0] = rs16_new
                            rs16 = rs16_holder[0]
                            nc.tensor.matmul(
                                rs16[:],
                                sel16_r[:, blk * NBLK:(blk + 1) * NBLK],
                                pt[:], start=(blk == 0),
                                stop=(blk == NBLK - 1))
                            if blk == NBLK - 1:
                                rsx = rsp.tile([NBLK, LBLK], f32, tag="rsx")
                                nc.vector.tensor_copy(rsx[:], rs16[:])
                                nc.sync.dma_start(rs_d[b], rsx[:])
                        op_t = ops.tile([128, LBLK], f32, tag="op")
                        nc.tensor.matmul(
                            op_t[:], v_r[:, ts(b, 128)], pt[:],
                            start=True, stop=True)
                        oc = ocp.tile([128, LBLK], out_dt, tag="oc")
                        oce = cfg["out_copy_engine"]
                        if oce == "alt":
                            oce = "act" if blk % 2 == 0 else "dve"
                        elif oce.startswith("mix"):
                            n, m = oce[3:].split("of")
                            oce = "dve" if blk % int(m) < int(n) else "act"
                        if oce == "act":
                            nc.scalar.copy(oc[:], op_t[:])
                        else:
                            nc.vector.tensor_copy(oc[:], op_t[:])
                        st = {"pool": nc.gpsimd, "act": nc.scalar,
                              "dve": nc.vector, "sp": nc.sync}[cfg["store_engine"]]
                        st.dma_start(outT_d[b, :, l0:l0 + LBLK], oc[:])
                    if cfg["rowsum_mode"] == "dve":
                        nc.gpsimd.dma_start(rs_d[b], rs_stage[:])
                    elif cfg["rowsum_mode"] == "pool":
                        nc.sync.dma_start(rs_d[b], rs_stage[0:1, :].rearrange("a b -> (a b)"))

            if repeat == 1:
                body()
            else:
                with tc.For_i(0, repeat, 1) as _i:
                    body(_i)

    nc.compile()
    return nc


def _make_runner(repeat=1, cfg=None):
    """Compile (once) and return fn(in_maps) -> list[dict] per core."""
    key = (repeat, tuple(sorted((cfg or {}).items())))
    if key in _RUNNER_CACHE:
        return _RUNNER_CACHE[key]

    import jax
    import concourse.mybir as mybir
    from concourse import bass2jax
    from concourse.bass2jax import _bass_exec_p, partition_id_tensor
    from jax.sharding import Mesh, NamedSharding, PartitionSpec
    from jax.experimental.shard_map import shard_map

    nc = _build_nc(repeat, cfg)
    bass2jax.install_neuronx_cc_hook()

    in_names, out_names, out_avals, zero_shapes = [], [], [], []
    for alloc in nc.m.functions[0].allocations:
        if not isinstance(alloc, mybir.MemoryLocationSet):
            continue
        name = alloc.memorylocations[0].name
        if alloc.kind == "ExternalInput":
            if nc.partition_id_tensor is None or name != nc.partition_id_tensor.name:
                in_names.append(name)
        elif alloc.kind == "ExternalOutput":
            out_names.append(name)
            shape = tuple(alloc.tensor_shape)
            dtype = mybir.dt.np(alloc.dtype)
            out_avals.append(jax.core.ShapedArray(shape, dtype))
            zero_shapes.append((shape, dtype))
    n_params = len(in_names)
    pid_name = nc.partition_id_tensor.name if nc.partition_id_tensor else None
    names_for_bind = in_names + out_names + ([pid_name] if pid_name else [])

    def _body(*args):
        operands = list(args)
        if pid_name:
            operands.append(partition_id_tensor())
        outs = _bass_exec_p.bind(
            *operands,
            out_avals=tuple(out_avals),
            in_names=tuple(names_for_bind),
            out_names=tuple(out_names),
            lowering_input_output_aliases=(),
            sim_require_finite=True,
            sim_require_nnan=True,
            nc=nc,
        )
        return tuple(outs)

    devices = jax.devices()[:N_CORES]
    mesh = Mesh(np.asarray(devices), ("core",))
    nspec = n_params + len(out_names)
    fn = jax.jit(
        shard_map(_body, mesh=mesh,
                  in_specs=(PartitionSpec("core"),) * nspec,
                  out_specs=(PartitionSpec("core"),) * len(out_names),
                  check_rep=False),
        keep_unused=True)
    sharding = NamedSharding(mesh, PartitionSpec("core"))

    def run(in_maps):
        import jax as _jax
        concat_in = [
            np.concatenate([np.asarray(m[name]) for m in in_maps], axis=0)
            for name in in_names
        ]
        zeros = [np.zeros((N_CORES * s[0],) + tuple(s[1:]), d)
                 for (s, d) in zero_shapes]
        dev_in = [_jax.device_put(a, sharding) for a in concat_in + zeros]
        out_arrs = fn(*dev_in)
        _jax.block_until_ready(out_arrs)
        return [
            {name: np.asarray(out_arrs[i]).reshape(
                (N_CORES,) + tuple(out_avals[i].shape))[c]
             for i, name in enumerate(out_names)}
            for c in range(N_CORES)
        ], (fn, dev_in)

    _RUNNER_CACHE[key] = run
    return run


def _prep_inputs_t2(queries, keys, values, cfg=None):
    cfg = {**CFG, **(cfg or {})}
    import ml_dtypes
    bf = ml_dtypes.bfloat16
    q = np.asarray(queries, np.float32)
    k = np.asarray(keys, np.float32)
    v = np.asarray(values, np.float32)
    mmdt = np.float16 if cfg["t2_mm_dtype"] == "f16" else bf
    qT16 = np.ascontiguousarray(q.transpose(0, 2, 1).astype(mmdt))
    k16 = np.ascontiguousarray(k.astype(mmdt))                 # (B, e, j)
    v16 = np.ascontiguousarray(v.astype(bf))                   # (B, j, d)
    qsum = q.sum(axis=2)                                       # (B, L) f32
    krow = k.sum(axis=2)                                       # (B, S) f32
    f16 = mmdt
    qh = qsum.astype(f16)
    ql = (qsum - qh.astype(np.float32)).astype(f16)
    kh = krow.astype(f16)
    kl = (krow - kh.astype(np.float32)).astype(f16)
    c = np.maximum(qsum * krow.max(axis=1)[:, None],
                   qsum * krow.min(axis=1)[:, None])           # (B, L)
    cb = c.astype(f16)
    ones = np.ones((B, S), np.float32).astype(f16)
    alhs = np.ascontiguousarray(
        np.stack([kh, kl, kh, -ones], axis=1))                 # (B, 4, S)
    arhs = np.ascontiguousarray(
        np.stack([qh, qh, ql, cb], axis=1))                    # (B, 4, L)
    sel = np.zeros((S, NBLK * NBLK), np.float32)
    for j in range(NBLK):
        sel[:, j * NBLK + j] = 1.0
    selb = sel.astype(bf)
    in_maps = []
    for ci in range(N_CORES):
        sl = slice(ci * B_LOC, (ci + 1) * B_LOC)
        in_maps.append({"qT16": qT16[sl], "k16": k16[sl], "v16": v16[sl],
                        "alhs": alhs[sl], "arhs": arhs[sl], "selb": selb,
                        "onesb": np.ones((S, 1), np.float32).astype(bf)})
    return in_maps


def _assemble_t2(results):
    out = np.empty((B, L, S), dtype=np.float32)
    for ci in range(N_CORES):
        outT = results[ci]["outT"]         # (B_LOC, S, L) bf16, = out^T
        rs = results[ci]["rs"]             # (B_LOC, NBLK, LBLK) f32
        for b in range(B_LOC):
            rsum = rs[b].reshape(L)
            out[ci * B_LOC + b] = outT[b].astype(np.float32).T / rsum[:, None]
    return out.reshape(B, 1, L, S)


def _prep_inputs(queries, keys, values, cfg=None):
    cfg = {**CFG, **(cfg or {})}
    if cfg["dataflow"] == "t2":
        return _prep_inputs_t2(queries, keys, values, cfg)
    qT = np.ascontiguousarray(queries.transpose(0, 2, 1))      # (B, E, L)
    kp = keys + keys.sum(axis=2)[:, None, :]                   # k' = k + 1*ksum
    kp = np.ascontiguousarray(kp.astype(np.float32))
    v = np.ascontiguousarray(values.astype(np.float32))
    qT2 = kph = kpl = None
    if cfg["mm1_dtype"] == "bf16x2":
        import ml_dtypes
        bf = ml_dtypes.bfloat16
        qTh = qT.astype(bf)
        qTl = (qT - qTh.astype(np.float32)).astype(bf)
        qT2 = np.ascontiguousarray(np.stack([qTh, qTl], axis=2))
        kph = kp.astype(bf)
        kpl = np.ascontiguousarray((kp - kph.astype(np.float32)).astype(bf))
        kph = np.ascontiguousarray(kph)
    ind = np.zeros((NT, LBLK), np.float32)
    for ti in range(NT):
        ind[ti, ti * 128:(ti + 1) * 128] = 1.0
    sel16 = np.zeros((S, NBLK * NBLK), np.float32)
    for j in range(NBLK):
        sel16[:, j * NBLK + j] = 1.0
    in_maps = []
    for c in range(N_CORES):
        sl = slice(c * B_LOC, (c + 1) * B_LOC)
        m = {"qT": qT[sl], "kp": kp[sl], "v": v[sl], "ind": ind,
             "sel16": sel16}
        if qT2 is not None:
            m.update({"qT2": qT2[sl], "kph": kph[sl], "kpl": kpl[sl]})
        in_maps.append(m)
    return in_maps


def _assemble(results, cfg=None):
    cfg = {**CFG, **(cfg or {})}
    if cfg["dataflow"] == "t2":
        return _assemble_t2(results)
    out = np.empty((B, L, S), dtype=np.float32)
    for c in range(N_CORES):
        outT = results[c]["outT"]          # (B_LOC, S, L)  = out^T per batch
        rs = results[c]["rs"]              # (B_LOC, 128, L//128) rowsums
        for b in range(B_LOC):
            if cfg["rowsum_mode"] == "dve" and cfg["dataflow"] == "lsoft":
                rsum = rs[b].T.reshape(L)  # rowsum[l]
            else:
                rsum = rs[b].reshape(L)
            out[c * B_LOC + b] = outT[b].T / rsum[:, None]
    return out.reshape(B, 1, L, S)


def kernel(queries, keys, values):
    run = _make_runner(repeat=1)
    in_maps = _prep_inputs(queries, keys, values)
    results, _ = run(in_maps)
    return _assemble(results)

